# revision 1
# baseline (speedup 1.0000x reference)
"""TimeSformer-style block (temporal attn -> spatial attn -> MLP) on 8 trn2 cores.

Data-parallel over B=8: each NeuronCore processes one batch element end to end.
Heavy linears run as fp32r (full-rate) matmuls with contraction on the partition
dim; attention score matmuls fold the block-diagonal mask into extra one-hot
contraction rows (+/-20^2 additive mask -> exp() kills cross-sequence terms).
"""

import os
import sys
from contextlib import ExitStack

sys.path.insert(0, "/opt/trn_rl_repo")

import numpy as np

import concourse.bass as bass
import concourse.mybir as mybir
import concourse.tile as tile
from concourse import bacc
from concourse.bass_utils import run_bass_kernel_spmd
from concourse.masks import make_identity

F32 = mybir.dt.float32
F32R = mybir.dt.float32r
AF = mybir.ActivationFunctionType
ALU = mybir.AluOpType

C = 768
CO = 6  # C / 128
H = 12
D = 64
T = 8
G = 196          # h*w sequences
NT = G * T       # 1568 temporal tokens
NSEQ = 197       # spatial seq len (cls + 196)
NS = 8 * NSEQ    # 1576 spatial tokens
N = 1569
HID = 3072
HIDO = 24        # HID / 128
P = 128
EPS = 1e-5
SQL = 20.0       # sqrt of additive mask magnitude (20^2 = 400)
SCALE = D ** -0.5

PHASES = os.environ.get("KPHASES", "12345678")
KDEBUG = bool(int(os.environ.get("KDEBUG", "0")))


def _ceil(a, b):
    return (a + b - 1) // b


def ln_tile(nc, tmp, out_pool, x_t, rows, eps_t):
    """LayerNorm over free dim (768) of a [rows<=128, 768] token-major tile."""
    stats = tmp.tile([P, 3, 6], F32, tag="ln_stats")
    for s in range(3):
        nc.vector.bn_stats(out=stats[:rows, s], in_=x_t[:rows, s * 256:(s + 1) * 256])
    mv = tmp.tile([P, 2], F32, tag="ln_mv")
    nc.vector.bn_aggr(out=mv[:rows], in_=stats[:rows])
    nc.scalar.activation(out=mv[:rows, 1:2], in_=mv[:rows, 1:2], func=AF.Sqrt,
                         bias=eps_t[:rows], scale=1.0)
    nc.vector.reciprocal(out=mv[:rows, 1:2], in_=mv[:rows, 1:2])
    xln = out_pool.tile([P, C], F32, tag="ln_out")
    nc.vector.tensor_scalar(out=xln[:rows], in0=x_t[:rows],
                            scalar1=mv[:rows, 0:1], scalar2=mv[:rows, 1:2],
                            op0=ALU.subtract, op1=ALU.mult)
    return xln


def qkv_common(nc, tc, ctx, src_rows_dma, ntok, w_qk, w_v, qk_img, v_img,
               ident, eps_t, evict_f32r):
    """LN -> PE transpose -> q,k (c-major) + v (token-major) -> DRAM images."""
    wpool = ctx.enter_context(tc.tile_pool(name="w_qkv", bufs=1))
    lnp = ctx.enter_context(tc.tile_pool(name="lnp", bufs=3))
    lnout = ctx.enter_context(tc.tile_pool(name="lnout", bufs=3))
    tpp = ctx.enter_context(tc.tile_pool(name="tp_psum", bufs=4, space="PSUM"))
    xlnT_pool = ctx.enter_context(tc.tile_pool(name="xlnT", bufs=1))
    mmp = ctx.enter_context(tc.tile_pool(name="qkv_psum", bufs=3, space="PSUM"))
    evp = ctx.enter_context(tc.tile_pool(name="qkv_ev", bufs=4))

    wqk = wpool.tile([P, CO, 2 * C], F32R)
    nc.sync.dma_start(wqk, w_qk[:].rearrange("(co p) o -> p co o", p=P))
    wv = wpool.tile([P, CO, C], F32R)
    nc.sync.dma_start(wv, w_v[:].rearrange("(co p) o -> p co o", p=P))

    xlnT = xlnT_pool.tile([P, CO, ntok], F32R)

    ntiles = _ceil(ntok, P)
    for i in range(ntiles):
        rows = min(P, ntok - i * P)
        x_t = lnp.tile([P, C], F32, tag="x_t")
        src_rows_dma(x_t, i, rows)
        xln = ln_tile(nc, lnp, lnout, x_t, rows, eps_t)
        for co in range(CO):
            pt = tpp.tile([P, P], F32, tag="tp_psum")
            nc.tensor.transpose(pt[:, :rows], xln[:rows, co * P:(co + 1) * P],
                                ident[:rows, :rows])
            nc.vector.tensor_copy(out=xlnT[:, co, i * P:i * P + rows],
                                  in_=pt[:, :rows])

    for b in range(_ceil(ntok, 512)):
        cols = min(512, ntok - b * 512)
        for o in range(12):  # 2C/128 output chunks (q then k)
            ps = mmp.tile([P, 512], F32, tag="mm_ps")
            for co in range(CO):
                nc.tensor.matmul(ps[:, :cols], wqk[:, co, o * P:(o + 1) * P],
                                 xlnT[:, co, b * 512:b * 512 + cols],
                                 start=(co == 0), stop=(co == CO - 1))
            dst = qk_img[o * P:(o + 1) * P, b * 512:b * 512 + cols]
            ev = evp.tile([P, 512], F32R if evict_f32r else F32, tag="qk_ev")
            nc.vector.tensor_copy(out=ev[:, :cols], in_=ps[:, :cols])
            nc.sync.dma_start(dst, ev[:, :cols])
    for i in range(ntiles):
        rows = min(P, ntok - i * P)
        for half, fcols in ((0, 512), (1, 256)):
            ps = mmp.tile([P, 512], F32, tag="mm_ps")
            for co in range(CO):
                nc.tensor.matmul(
                    ps[:rows, :fcols], xlnT[:, co, i * P:i * P + rows],
                    wv[:, co, half * 512:half * 512 + fcols],
                    start=(co == 0), stop=(co == CO - 1))
            dst = v_img[i * P:i * P + rows, half * 512:half * 512 + fcols]
            ev = evp.tile([P, 512], F32R if evict_f32r else F32, tag="v_ev")
            nc.vector.tensor_copy(out=ev[:rows, :fcols], in_=ps[:rows, :fcols])
            nc.sync.dma_start(dst, ev[:rows, :fcols])


def phase2_temporal_attn(nc, tc, qk_t, v_t, aux_k_t, aux_q_t, sel_sb, oT_t):
    with ExitStack() as ctx:
        stg = ctx.enter_context(tc.tile_pool(name="t_stage", bufs=2))
        vstg = ctx.enter_context(tc.tile_pool(name="t_vstage", bufs=2))
        sp = ctx.enter_context(tc.tile_pool(name="t_spsum", bufs=2, space="PSUM"))
        op = ctx.enter_context(tc.tile_pool(name="t_opsum", bufs=2, space="PSUM"))
        pp = ctx.enter_context(tc.tile_pool(name="t_p", bufs=3))
        sig = ctx.enter_context(tc.tile_pool(name="t_sig", bufs=1))
        bcp = ctx.enter_context(tc.tile_pool(name="t_bc", bufs=2, space="PSUM"))

        sigma = sig.tile([12, NT], F32)
        rinv = sig.tile([12, NT], F32R)
        nst = _ceil(NT, P)  # 13 subtiles (12x128 + 32)
        for h in range(H):
            kq = stg.tile([P, 2, NT], F32, tag="kq")  # [0:64] data, [64:81] aux
            nc.sync.dma_start(kq[0:D, 0, :], qk_t[C + h * D:C + (h + 1) * D, :])
            nc.sync.dma_start(kq[0:D, 1, :], qk_t[h * D:(h + 1) * D, :])
            nc.sync.dma_start(kq[D:D + 17, 0, :], aux_k_t[:])
            nc.sync.dma_start(kq[D:D + 17, 1, :], aux_q_t[:])
            va = vstg.tile([P, nst, D + 1], F32, tag="va")
            nc.vector.memset(va, 1.0)
            for st in range(nst):
                rows = min(P, NT - st * P)
                nc.sync.dma_start(va[:rows, st, 0:D],
                                  v_t[st * P:st * P + rows, h * D:(h + 1) * D])
            for st in range(nst):
                rows = min(P, NT - st * P)
                ps = sp.tile([P, P], F32, tag="s_ps")
                nc.tensor.matmul(ps[:rows, :rows],
                                 kq[0:D + 17, 0, st * P:st * P + rows],
                                 kq[0:D + 17, 1, st * P:st * P + rows],
                                 start=True, stop=True)
                p_t = pp.tile([P, P], F32, tag="p_t")
                nc.scalar.activation(out=p_t[:rows, :rows], in_=ps[:rows, :rows],
                                     func=AF.Exp, scale=SCALE)
                po = op.tile([D + 1, P], F32, tag="o_ps")
                nc.tensor.matmul(po[:, :rows], va[:rows, st, :], p_t[:rows, :rows],
                                 start=True, stop=True)
                nc.vector.tensor_copy(
                    out=oT_t[(h % 2) * D:(h % 2) * D + D, h // 2,
                             st * P:st * P + rows],
                    in_=po[0:D, :rows])
                sgst = pp.tile([1, P], F32, tag="sg_st")
                nc.vector.tensor_copy(out=sgst[:, :rows], in_=po[D:D + 1, :rows])
                nc.sync.dma_start(sigma[h:h + 1, st * P:st * P + rows],
                                  sgst[:, :rows])
        with nc.allow_low_precision(reason="rinv feeds fp32r bcast matmul"):
            nc.vector.reciprocal(out=rinv, in_=sigma)
        for pr in range(6):
            for b in range(_ceil(NT, 512)):
                cols = min(512, NT - b * 512)
                bc = bcp.tile([P, 512], F32, tag="bc_ps")
                nc.tensor.matmul(bc[:, :cols], sel_sb[:, pr * P:(pr + 1) * P],
                                 rinv[:, b * 512:b * 512 + cols],
                                 start=True, stop=True)
                nc.vector.tensor_mul(
                    out=oT_t[:, pr, b * 512:b * 512 + cols],
                    in0=oT_t[:, pr, b * 512:b * 512 + cols],
                    in1=bc[:, :cols])


def phase3_temporal_proj(nc, tc, x_in, w_proj_t, w_tfc, oT_t, xs_d):
    with ExitStack() as ctx:
        wp = ctx.enter_context(tc.tile_pool(name="p3_w", bufs=1))
        mp = ctx.enter_context(tc.tile_pool(name="p3_ps", bufs=4, space="PSUM"))
        pj = ctx.enter_context(tc.tile_pool(name="p3_pj", bufs=2))
        tp = ctx.enter_context(tc.tile_pool(name="p3_t", bufs=3))

        wproj = wp.tile([P, CO, C], F32R)
        nc.sync.dma_start(wproj, w_proj_t[:].rearrange("(co p) o -> p co o", p=P))
        wtfc = wp.tile([P, CO, C], F32R)
        nc.sync.dma_start(wtfc, w_tfc[:].rearrange("(co p) o -> p co o", p=P))

        # xs_d[f, 0, :] = x[0] (cls) for every frame
        cls_sb = tp.tile([8, C], F32, tag="cls_sb")
        nc.gpsimd.dma_start(cls_sb, bass.AP(tensor=x_in, offset=0,
                                            ap=[[0, 8], [1, C]]))
        nc.sync.dma_start(xs_d[:, 0, :], cls_sb)

        for b in range(_ceil(NT, 512)):
            cols = min(512, NT - b * 512)
            projT = pj.tile([P, CO, 512], F32R, tag="projT")
            for o in range(CO):
                ps = mp.tile([P, 512], F32, tag="p3ps")
                for co in range(CO):
                    nc.tensor.matmul(ps[:, :cols], wproj[:, co, o * P:(o + 1) * P],
                                     oT_t[:, co, b * 512:b * 512 + cols],
                                     start=(co == 0), stop=(co == CO - 1))
                nc.vector.tensor_copy(out=projT[:, o, :cols], in_=ps[:, :cols])
            for it in range(_ceil(cols, P)):
                tok0 = b * 512 + it * P
                rows = min(P, NT - tok0)
                x_t = tp.tile([P, C], F32, tag="x_t3")
                nc.sync.dma_start(x_t[:rows], x_in[1 + tok0:1 + tok0 + rows, :])
                xt = tp.tile([P, C], F32, tag="xt3")
                for half, fcols in ((0, 512), (1, 256)):
                    ps = mp.tile([P, 512], F32, tag="p3ps")
                    for co in range(CO):
                        nc.tensor.matmul(
                            ps[:rows, :fcols], projT[:, co, it * P:it * P + rows],
                            wtfc[:, co, half * 512:half * 512 + fcols],
                            start=(co == 0), stop=(co == CO - 1))
                    nc.vector.tensor_add(
                        out=xt[:rows, half * 512:half * 512 + fcols],
                        in0=ps[:rows, :fcols],
                        in1=x_t[:rows, half * 512:half * 512 + fcols])
                # token g*8+t -> xs_d[t, 1+g]; permutation on the DRAM-side AP
                g0 = tok0 // T
                ng = rows // T
                nc.sync.dma_start(
                    xs_d[:, 1 + g0:1 + g0 + ng, :].rearrange("t g c -> g t c"),
                    xt[:rows])


def phase5_spatial_attn(nc, tc, qk_s, v_s, aux_k_s, aux_q_s, sel_sb, oT_s):
    with ExitStack() as ctx:
        stg = ctx.enter_context(tc.tile_pool(name="s_stage", bufs=2))
        vstg = ctx.enter_context(tc.tile_pool(name="s_vstage", bufs=2))
        sp = ctx.enter_context(tc.tile_pool(name="s_spsum", bufs=3, space="PSUM"))
        op = ctx.enter_context(tc.tile_pool(name="s_opsum", bufs=2, space="PSUM"))
        pp = ctx.enter_context(tc.tile_pool(name="s_p", bufs=5))
        sig = ctx.enter_context(tc.tile_pool(name="s_sig", bufs=2))
        bcp = ctx.enter_context(tc.tile_pool(name="s_bc", bufs=2, space="PSUM"))

        W2 = 2 * NSEQ  # 394
        for fp in range(4):  # frame pairs (2fp, 2fp+1)
            col0 = fp * W2
            qs = stg.tile([P, 12, W2], F32R, tag="qs")
            ks = stg.tile([P, 12, W2], F32R, tag="ks")
            for h in range(H):
                nc.sync.dma_start(qs[0:D, h], qk_s[h * D:(h + 1) * D,
                                                   col0:col0 + W2])
                nc.sync.dma_start(ks[0:D, h], qk_s[C + h * D:C + (h + 1) * D,
                                                   col0:col0 + W2])
            nc.gpsimd.dma_start(
                qs[D:D + 3], bass.AP(tensor=aux_q_s, offset=col0,
                                     ap=[[NS, 3], [0, 12], [1, W2]]))
            nc.gpsimd.dma_start(
                ks[D:D + 3], bass.AP(tensor=aux_k_s, offset=col0,
                                     ap=[[NS, 3], [0, 12], [1, W2]]))
            vs = vstg.tile([P, 12, 2, 2, D + 1], F32R, tag="vs")
            nc.vector.memset(vs.bitcast(F32), 1.0)
            for h in range(H):
                for f in range(2):
                    for ch, chlen in ((0, P), (1, NSEQ - P)):
                        r0 = (2 * fp + f) * NSEQ + ch * P
                        nc.sync.dma_start(vs[0:chlen, h, f, ch, 0:D],
                                          v_s[r0:r0 + chlen, h * D:(h + 1) * D])
            sigma = sig.tile([12, W2], F32, tag="sigma_s")
            rinv = sig.tile([12, W2], F32R, tag="rinv_s")
            for h in range(H):
                po = op.tile([D + 1, W2], F32, tag="o_ps_s")
                passes = [(f, ch, chlen) for f in range(2)
                          for ch, chlen in ((0, P), (1, NSEQ - P))]
                for idx, (f, ch, chlen) in enumerate(passes):
                    ccol = f * NSEQ + ch * P
                    ps = sp.tile([P, W2], F32, tag="s_ps_s")
                    nc.tensor.matmul(ps[:chlen],
                                     ks[0:D + 3, h, ccol:ccol + chlen],
                                     qs[0:D + 3, h], start=True, stop=True)
                    p_t = pp.tile([P, W2], F32R, tag="p_s")
                    nc.scalar.activation(out=p_t[:chlen], in_=ps[:chlen],
                                         func=AF.Exp, scale=SCALE)
                    nc.tensor.matmul(po, vs[0:chlen, h, f, ch], p_t[:chlen],
                                     start=(idx == 0), stop=(idx == 3))
                nc.vector.tensor_copy(
                    out=oT_s[(h % 2) * D:(h % 2) * D + D, h // 2,
                             col0:col0 + W2],
                    in_=po[0:D])
                sgst = pp.tile([1, W2], F32, tag="sg_st")
                nc.vector.tensor_copy(out=sgst, in_=po[D:D + 1])
                nc.sync.dma_start(sigma[h:h + 1], sgst)
            with nc.allow_low_precision(reason="rinv feeds fp32r bcast matmul"):
                nc.vector.reciprocal(out=rinv, in_=sigma)
            for pr in range(6):
                bc = bcp.tile([P, W2], F32, tag="bc_s")
                nc.tensor.matmul(bc, sel_sb[:, pr * P:(pr + 1) * P], rinv,
                                 start=True, stop=True)
                nc.vector.tensor_mul(out=oT_s[:, pr, col0:col0 + W2],
                                     in0=oT_s[:, pr, col0:col0 + W2], in1=bc)


def phase6_spatial_proj(nc, tc, xs_d, w_proj_s, oT_s, y_s):
    with ExitStack() as ctx:
        wp = ctx.enter_context(tc.tile_pool(name="p6_w", bufs=1))
        mp = ctx.enter_context(tc.tile_pool(name="p6_ps", bufs=4, space="PSUM"))
        tp = ctx.enter_context(tc.tile_pool(name="p6_t", bufs=3))
        wproj = wp.tile([P, CO, C], F32R)
        nc.sync.dma_start(wproj, w_proj_s[:].rearrange("(co p) o -> p co o", p=P))
        xs_flat = xs_d[:].rearrange("f n c -> (f n) c")
        ys_flat = y_s[:].rearrange("f n c -> (f n) c")
        for i in range(_ceil(NS, P)):
            rows = min(P, NS - i * P)
            x_t = tp.tile([P, C], F32, tag="x_t6")
            nc.sync.dma_start(x_t[:rows], xs_flat[i * P:i * P + rows, :])
            yt = tp.tile([P, C], F32, tag="yt6")
            for half, fcols in ((0, 512), (1, 256)):
                ps = mp.tile([P, 512], F32, tag="p6_psA")
                for co in range(CO):
                    nc.tensor.matmul(
                        ps[:rows, :fcols], oT_s[:, co, i * P:i * P + rows],
                        wproj[:, co, half * 512:half * 512 + fcols],
                        start=(co == 0), stop=(co == CO - 1))
                nc.vector.tensor_add(
                    out=yt[:rows, half * 512:half * 512 + fcols],
                    in0=ps[:rows, :fcols],
                    in1=x_t[:rows, half * 512:half * 512 + fcols])
            nc.sync.dma_start(ys_flat[i * P:i * P + rows, :], yt[:rows])


def phase7_cls(nc, tc, y_s, one8, cls_row):
    with ExitStack() as ctx:
        tp = ctx.enter_context(tc.tile_pool(name="p7_t", bufs=1))
        mp = ctx.enter_context(tc.tile_pool(name="p7_ps", bufs=2, space="PSUM"))
        ycls = tp.tile([8, C], F32)
        nc.sync.dma_start(ycls, y_s[:, 0, :])
        o8 = tp.tile([8, 1], F32)
        nc.sync.dma_start(o8, one8[:])
        res = tp.tile([1, C], F32)
        for half, fcols in ((0, 512), (1, 256)):
            ps = mp.tile([1, 512], F32, tag="p7_ps")
            nc.tensor.matmul(ps[:, :fcols], o8,
                             ycls[:, half * 512:half * 512 + fcols],
                             start=True, stop=True)
            nc.vector.tensor_copy(out=res[:, half * 512:half * 512 + fcols],
                                  in_=ps[:, :fcols])
        nc.sync.dma_start(cls_row[:], res)


def phase8_mlp(nc, tc, y_s, cls_row, w_fc1, w_fc2, h1_img, out, ident, eps_t):
    blocks = [(0, 512), (512, 512), (1024, 512), (1536, 33)]

    def load_x(pool, tok0, it, btok):
        rows = min(P, btok - it * P)
        x_t = pool.tile([P, C], F32, tag="x_t8")
        g0 = (tok0 + it * P) // T
        if btok == 33:
            nc.sync.dma_start(
                x_t[:32],
                y_s[:, 1 + g0:1 + g0 + 4, :].rearrange("t g c -> g t c"))
            nc.sync.dma_start(x_t[32:33], cls_row[:])
        else:
            nc.sync.dma_start(
                x_t[:rows],
                y_s[:, 1 + g0:1 + g0 + 16, :].rearrange("t g c -> g t c"))
        return x_t, rows

    # pass A: h1 = gelu(fc1(ln(y))) -> h1 image [HID, N'] (c-major)
    with ExitStack() as ctx:
        wp = ctx.enter_context(tc.tile_pool(name="p8a_w", bufs=1))
        lnp = ctx.enter_context(tc.tile_pool(name="p8a_ln", bufs=3))
        lnout = ctx.enter_context(tc.tile_pool(name="p8a_lno", bufs=3))
        tpp = ctx.enter_context(tc.tile_pool(name="p8a_tp", bufs=2, space="PSUM"))
        xlp = ctx.enter_context(tc.tile_pool(name="p8a_xlT", bufs=2))
        mp = ctx.enter_context(tc.tile_pool(name="p8a_ps", bufs=4, space="PSUM"))
        evp = ctx.enter_context(tc.tile_pool(name="p8a_ev", bufs=4))

        wfc1 = wp.tile([P, CO, HID], F32R)
        nc.sync.dma_start(wfc1, w_fc1[:].rearrange("(co p) o -> p co o", p=P))
        for tok0, btok in blocks:
            xlnT = xlp.tile([P, CO, 512], F32R, tag="xlnT8")
            if btok % 2:
                nc.vector.memset(xlnT.bitcast(F32), 0.0)
            for it in range(_ceil(btok, P)):
                x_t, rows = load_x(lnp, tok0, it, btok)
                xln = ln_tile(nc, lnp, lnout, x_t, rows, eps_t)
                for co in range(CO):
                    pt = tpp.tile([P, P], F32, tag="tp8")
                    nc.tensor.transpose(pt[:, :rows],
                                        xln[:rows, co * P:(co + 1) * P],
                                        ident[:rows, :rows])
                    nc.vector.tensor_copy(out=xlnT[:, co, it * P:it * P + rows],
                                          in_=pt[:, :rows])
            btok_mm = btok + (btok % 2)
            for o in range(HIDO):
                ps = mp.tile([P, 512], F32, tag="p8ps")
                for co in range(CO):
                    nc.tensor.matmul(ps[:, :btok_mm], wfc1[:, co, o * P:(o + 1) * P],
                                     xlnT[:, co, :btok_mm],
                                     start=(co == 0), stop=(co == CO - 1))
                ev = evp.tile([P, 512], F32R, tag="h1ev")
                nc.scalar.activation(out=ev[:, :btok], in_=ps[:, :btok],
                                     func=AF.Gelu)
                nc.sync.dma_start(h1_img[o * P:(o + 1) * P, tok0:tok0 + btok],
                                  ev[:, :btok])

    # pass B: out = y + fc2(h1)
    with ExitStack() as ctx:
        wp = ctx.enter_context(tc.tile_pool(name="p8b_w", bufs=1))
        hp = ctx.enter_context(tc.tile_pool(name="p8b_h", bufs=3))
        xrp = ctx.enter_context(tc.tile_pool(name="p8b_xr", bufs=3))
        mp = ctx.enter_context(tc.tile_pool(name="p8b_ps", bufs=4, space="PSUM"))
        otp = ctx.enter_context(tc.tile_pool(name="p8b_o", bufs=3))

        wfc2 = wp.tile([P, HIDO, C], F32R)
        nc.sync.dma_start(wfc2, w_fc2[:].rearrange("(ho p) o -> p ho o", p=P))
        for tok0, btok in blocks:
            for it in range(_ceil(btok, P)):
                x_t, rows = load_x(xrp, tok0, it, btok)
                h1t = hp.tile([P, HIDO, P], F32R, tag="h1t")
                nc.sync.dma_start(
                    h1t[:, :, :rows],
                    h1_img[:, tok0 + it * P:tok0 + it * P + rows].rearrange(
                        "(ho p) n -> p ho n", p=P))
                o_t = otp.tile([P, C], F32, tag="o_t8")
                for half, fcols in ((0, 512), (1, 256)):
                    ps = mp.tile([P, 512], F32, tag="p8ps")
                    for ho in range(HIDO):
                        nc.tensor.matmul(
                            ps[:rows, :fcols], h1t[:, ho, :rows],
                            wfc2[:, ho, half * 512:half * 512 + fcols],
                            start=(ho == 0), stop=(ho == HIDO - 1))
                    nc.vector.tensor_add(
                        out=o_t[:rows, half * 512:half * 512 + fcols],
                        in0=ps[:rows, :fcols],
                        in1=x_t[:rows, half * 512:half * 512 + fcols])
                row0 = tok0 + it * P
                if btok == 33:
                    nc.sync.dma_start(out[1 + row0:1 + row0 + 32, :], o_t[:32])
                    nc.sync.dma_start(out[0:1, :], o_t[32:33])
                else:
                    nc.sync.dma_start(out[1 + row0:1 + row0 + rows, :],
                                      o_t[:rows])


def build_nc():
    nc = bacc.Bacc("TRN2", target_bir_lowering=False, debug=False)

    x_in = nc.dram_tensor("x", (N, C), F32, kind="ExternalInput")
    w_qk_t = nc.dram_tensor("w_qk_t", (C, 2 * C), F32R, kind="ExternalInput")
    w_v_t = nc.dram_tensor("w_v_t", (C, C), F32R, kind="ExternalInput")
    w_qk_s = nc.dram_tensor("w_qk_s", (C, 2 * C), F32R, kind="ExternalInput")
    w_v_s = nc.dram_tensor("w_v_s", (C, C), F32R, kind="ExternalInput")
    w_proj_t = nc.dram_tensor("w_proj_t", (C, C), F32R, kind="ExternalInput")
    w_tfc = nc.dram_tensor("w_tfc", (C, C), F32R, kind="ExternalInput")
    w_proj_s = nc.dram_tensor("w_proj_s", (C, C), F32R, kind="ExternalInput")
    w_fc1 = nc.dram_tensor("w_fc1", (C, HID), F32R, kind="ExternalInput")
    w_fc2 = nc.dram_tensor("w_fc2", (HID, C), F32R, kind="ExternalInput")
    aux_k_t = nc.dram_tensor("aux_k_t", (17, NT), F32, kind="ExternalInput")
    aux_q_t = nc.dram_tensor("aux_q_t", (17, NT), F32, kind="ExternalInput")
    aux_k_s = nc.dram_tensor("aux_k_s", (3, NS), F32R, kind="ExternalInput")
    aux_q_s = nc.dram_tensor("aux_q_s", (3, NS), F32R, kind="ExternalInput")
    sel12 = nc.dram_tensor("sel12", (12, C), F32R, kind="ExternalInput")
    one8 = nc.dram_tensor("one8", (8, 1), F32, kind="ExternalInput")
    out = nc.dram_tensor("out", (N, C), F32, kind="ExternalOutput")
    dbg = {}
    if KDEBUG:
        for nm, shp in (("d_qk_t", (2 * C, NT)), ("d_v_t", (NT, C)),
                        ("d_oT_t", (P, CO, NT)), ("d_xs", (8, NSEQ, C)),
                        ("d_qk_s", (2 * C, NS)), ("d_v_s", (NS, C)),
                        ("d_oT_s", (P, CO, NS)), ("d_y_s", (8, NSEQ, C)),
                        ("d_cls", (1, C))):
            dbg[nm] = nc.dram_tensor(nm, shp, F32, kind="ExternalOutput")

    with tile.TileContext(nc) as tc:
        with tc.tile_pool(name="dram", bufs=1, space="DRAM") as dram, \
             tc.tile_pool(name="const", bufs=1) as const:
            qk_t = dram.tile([2 * C, NT], F32)
            v_t = dram.tile([NT, C], F32)
            xs_d = dram.tile([8, NSEQ, C], F32)
            qk_s = dram.tile([2 * C, NS], F32R)
            v_s = dram.tile([NS, C], F32R)
            y_s = dram.tile([8, NSEQ, C], F32)
            cls_row = dram.tile([1, C], F32)
            h1_img = dram.tile([HID, NT + 33], F32R)

            ident = const.tile([P, P], F32)
            make_identity(nc, ident)
            eps_t = const.tile([P, 1], F32)
            nc.vector.memset(eps_t, EPS)
            sel_sb = const.tile([12, C], F32R)
            nc.sync.dma_start(sel_sb, sel12[:])

            if "1" in PHASES:
                with ExitStack() as ctx:
                    def src_t(x_t, i, rows):
                        nc.sync.dma_start(x_t[:rows],
                                          x_in[1 + i * P:1 + i * P + rows, :])
                    qkv_common(nc, tc, ctx, src_t, NT, w_qk_t, w_v_t,
                               qk_t[:], v_t[:], ident, eps_t, evict_f32r=False)
            if "2" in PHASES or "3" in PHASES:
                with tc.tile_pool(name="oT_t_pool", bufs=1) as otp_:
                    oT_t = otp_.tile([P, CO, NT], F32R)
                    if "2" in PHASES:
                        phase2_temporal_attn(nc, tc, qk_t[:], v_t[:],
                                             aux_k_t, aux_q_t, sel_sb, oT_t)
                    if "3" in PHASES:
                        phase3_temporal_proj(nc, tc, x_in, w_proj_t, w_tfc,
                                             oT_t, xs_d)
                    if KDEBUG and "2" in PHASES:
                        nc.sync.dma_start(dbg["d_oT_t"][:], oT_t)
            if "4" in PHASES:
                with ExitStack() as ctx:
                    xs_flat = xs_d[:].rearrange("f n c -> (f n) c")

                    def src_s(x_t, i, rows):
                        nc.sync.dma_start(x_t[:rows],
                                          xs_flat[i * P:i * P + rows, :])
                    qkv_common(nc, tc, ctx, src_s, NS, w_qk_s, w_v_s,
                               qk_s[:], v_s[:], ident, eps_t, evict_f32r=True)
            if "5" in PHASES or "6" in PHASES:
                with tc.tile_pool(name="oT_s_pool", bufs=1) as osp_:
                    oT_s = osp_.tile([P, CO, NS], F32R)
                    if "5" in PHASES:
                        phase5_spatial_attn(nc, tc, qk_s[:], v_s[:],
                                            aux_k_s, aux_q_s, sel_sb, oT_s)
                    if "6" in PHASES:
                        phase6_spatial_proj(nc, tc, xs_d, w_proj_s, oT_s, y_s)
                    if KDEBUG and "5" in PHASES:
                        nc.sync.dma_start(dbg["d_oT_s"][:], oT_s)
            if "7" in PHASES:
                phase7_cls(nc, tc, y_s, one8, cls_row)
            if "8" in PHASES:
                phase8_mlp(nc, tc, y_s, cls_row, w_fc1, w_fc2, h1_img[:],
                           out, ident, eps_t)
            if KDEBUG:
                nc.sync.dma_start(dbg["d_qk_t"][:], qk_t[:])
                nc.sync.dma_start(dbg["d_v_t"][:], v_t[:])
                nc.sync.dma_start(dbg["d_xs"][:], xs_d[:])
                nc.sync.dma_start(dbg["d_qk_s"][:],
                                  qk_s[:].bitcast(F32))
                nc.sync.dma_start(dbg["d_v_s"][:], v_s[:].bitcast(F32))
                nc.sync.dma_start(dbg["d_y_s"][:], y_s[:])
                nc.sync.dma_start(dbg["d_cls"][:], cls_row[:])

    nc.compile()
    return nc


_NC_CACHE = None


def _get_nc():
    global _NC_CACHE
    if _NC_CACHE is None:
        _NC_CACHE = build_nc()
    return _NC_CACHE


def make_consts():
    gidx = (np.arange(NT) // T) % 16
    aux_k_t = np.zeros((17, NT), np.float32)
    aux_k_t[gidx, np.arange(NT)] = SQL
    aux_k_t[16, :] = SQL
    aux_q_t = aux_k_t.copy()
    aux_q_t[16, :] = -SQL

    fpar = (np.arange(NS) // NSEQ) % 2
    aux_k_s = np.zeros((3, NS), np.float32)
    aux_k_s[fpar, np.arange(NS)] = SQL
    aux_k_s[2, :] = SQL
    aux_q_s = aux_k_s.copy()
    aux_q_s[2, :] = -SQL

    sel = np.zeros((12, C), np.float32)
    for pr in range(6):
        for p in range(P):
            sel[2 * pr + p // D, pr * P + p] = 1.0
    one8 = np.full((8, 1), 0.125, np.float32)
    return aux_k_t, aux_q_t, aux_k_s, aux_q_s, sel, one8


def kernel(**inputs):
    x = np.ascontiguousarray(np.asarray(inputs["x"], dtype=np.float32))
    B = x.shape[0]
    tr = lambda w: np.ascontiguousarray(np.asarray(w, np.float32).T)
    qkv_w = np.asarray(inputs["qkv_w"], np.float32)
    tqkv_w = np.asarray(inputs["tqkv_w"], np.float32)
    aux_k_t, aux_q_t, aux_k_s, aux_q_s, sel, one8 = make_consts()
    shared = {
        "w_qk_t": tr(tqkv_w[:2 * C]), "w_v_t": tr(tqkv_w[2 * C:]),
        "w_qk_s": tr(qkv_w[:2 * C]), "w_v_s": tr(qkv_w[2 * C:]),
        "w_proj_t": tr(inputs["tproj_w"]), "w_tfc": tr(inputs["tfc_w"]),
        "w_proj_s": tr(inputs["proj_w"]),
        "w_fc1": tr(inputs["fc1_w"]), "w_fc2": tr(inputs["fc2_w"]),
        "aux_k_t": aux_k_t, "aux_q_t": aux_q_t,
        "aux_k_s": aux_k_s, "aux_q_s": aux_q_s,
        "sel12": sel, "one8": one8,
    }
    nc = _get_nc()
    in_maps = [dict(shared, x=np.ascontiguousarray(x[b])) for b in range(B)]
    res = run_bass_kernel_spmd(nc, in_maps, core_ids=list(range(B)),
                               trace=bool(int(os.environ.get("KTRACE", "0"))))
    out = np.stack([res.results[b]["out"] for b in range(B)], axis=0)
    kernel.last_results = res
    return out



# revision 13
# speedup vs baseline: 1.5727x; 1.5727x over previous
"""TimeSformer-style block (temporal attn -> spatial attn -> MLP) on 8 trn2 cores.

Data-parallel over B=8: each NeuronCore processes one batch element end to end.
All GEMMs run in bf16 (1 cycle/row on the PE at any free-dim size, 4x cheaper
weight loads than fp32r); PSUM accumulation stays fp32. Attention q/k/v images
live in SBUF (no HBM round trips); the MLP is fused per 512-token block so the
fc1 activation image never touches DRAM. Block-diagonal attention masking uses
a multiplicative 0/1 mask after exp instead of extra contraction rows.
"""

import os
import sys
from contextlib import ExitStack

sys.path.insert(0, "/opt/trn_rl_repo")

import numpy as np
import ml_dtypes

import concourse.bass as bass
import concourse.mybir as mybir
import concourse.tile as tile
from concourse import bacc
from concourse.bass_utils import run_bass_kernel_spmd

F32 = mybir.dt.float32
BF = mybir.dt.bfloat16
AF = mybir.ActivationFunctionType
ALU = mybir.AluOpType

C = 768
CO = 6           # C / 128
H = 12
D = 64
T = 8
G = 196          # h*w sequences
NT = G * T       # 1568 temporal tokens
NSEQ = 197       # spatial seq len (cls + 196)
NS = 8 * NSEQ    # 1576 spatial tokens
N = 1569
HID = 3072
HIDO = 24        # HID / 128
P = 128
EPS = 1e-5
SCALE = D ** -0.5

PHASES = os.environ.get("KPHASES", "12345678")
KDEBUG = bool(int(os.environ.get("KDEBUG", "0")))


def _ceil(a, b):
    return (a + b - 1) // b


def ln_tile(nc, tmp, out_pool, x_t, rows, eps_t):
    """LayerNorm over free dim (768) of a [rows<=128, 768] token-major tile."""
    stats = tmp.tile([P, 3, 6], F32, tag="ln_stats")
    for s in range(3):
        nc.vector.bn_stats(out=stats[:rows, s], in_=x_t[:rows, s * 256:(s + 1) * 256])
    mv = tmp.tile([P, 2], F32, tag="ln_mv")
    nc.vector.bn_aggr(out=mv[:rows], in_=stats[:rows])
    nc.scalar.activation(out=mv[:rows, 1:2], in_=mv[:rows, 1:2], func=AF.Sqrt,
                         bias=eps_t[:rows], scale=1.0)
    nc.vector.reciprocal(out=mv[:rows, 1:2], in_=mv[:rows, 1:2])
    xln = out_pool.tile([P, C], BF, tag="ln_out")
    nc.vector.tensor_scalar(out=xln[:rows], in0=x_t[:rows],
                            scalar1=mv[:rows, 0:1], scalar2=mv[:rows, 1:2],
                            op0=ALU.subtract, op1=ALU.mult)
    return xln


def qkv_phase(nc, tc, ctx, src_dma, ntok, wqk, wv, qk_img, v_img, vchunks,
              ident, eps_t):
    """LN -> PE transpose -> qk (c-major SBUF image) + v (token-major chunks)."""
    lnp = ctx.enter_context(tc.tile_pool(name="lnp", bufs=3))
    lnout = ctx.enter_context(tc.tile_pool(name="lnout", bufs=3))
    tpp = ctx.enter_context(tc.tile_pool(name="tp_ps", bufs=3, space="PSUM"))
    xlp = ctx.enter_context(tc.tile_pool(name="xlnT", bufs=1))
    mmp = ctx.enter_context(tc.tile_pool(name="mm_ps", bufs=4, space="PSUM"))

    xlnT = xlp.tile([P, CO, ntok], BF)
    ntiles = _ceil(ntok, P)
    for i in range(ntiles):
        rows = min(P, ntok - i * P)
        x_t = lnp.tile([P, C], BF, tag="x_t")
        src_dma(x_t, i, rows)
        xln = ln_tile(nc, lnp, lnout, x_t, rows, eps_t)
        for co in range(CO):
            pt = tpp.tile([P, P], BF, tag="tp")
            nc.tensor.transpose(pt[:, :rows], xln[:rows, co * P:(co + 1) * P],
                                ident[:rows, :rows])
            nc.vector.tensor_copy(out=xlnT[:, co, i * P:i * P + rows],
                                  in_=pt[:, :rows])
    for b in range(_ceil(ntok, 512)):
        cols = min(512, ntok - b * 512)
        for o in range(12):  # 2C/128 output chunks (q then k)
            ps = mmp.tile([P, 512], F32, tag="mm")
            for co in range(CO):
                nc.tensor.matmul(ps[:, :cols], wqk[:, co, o * P:(o + 1) * P],
                                 xlnT[:, co, b * 512:b * 512 + cols],
                                 start=(co == 0), stop=(co == CO - 1))
            nc.vector.tensor_copy(out=qk_img[:, o, b * 512:b * 512 + cols],
                                  in_=ps[:, :cols])
    for dst_fn, col0v, chlen in vchunks:
        for half, fcols in ((0, 512), (1, 256)):
            ps = mmp.tile([P, 512], F32, tag="mm")
            for co in range(CO):
                nc.tensor.matmul(ps[:chlen, :fcols],
                                 xlnT[:, co, col0v:col0v + chlen],
                                 wv[:, co, half * 512:half * 512 + fcols],
                                 start=(co == 0), stop=(co == CO - 1))
            nc.vector.tensor_copy(out=dst_fn(chlen, half), in_=ps[:chlen, :fcols])


def phase2_temporal_attn(nc, tc, qk_t, v_t, m01, sel_sb, oT_t):
    with ExitStack() as ctx:
        sp = ctx.enter_context(tc.tile_pool(name="t_sps", bufs=2, space="PSUM"))
        op = ctx.enter_context(tc.tile_pool(name="t_ops", bufs=3, space="PSUM"))
        pp = ctx.enter_context(tc.tile_pool(name="t_p", bufs=6))
        sig = ctx.enter_context(tc.tile_pool(name="t_sig", bufs=1))
        bcp = ctx.enter_context(tc.tile_pool(name="t_bc", bufs=2, space="PSUM"))

        sigma = sig.tile([12, NT], F32)
        rinv = sig.tile([12, NT], BF)
        nst = _ceil(NT, P)
        for h in range(H):
            hp, hc = (h % 2) * D, h // 2
            for st in range(nst):
                rows = min(P, NT - st * P)
                ps = sp.tile([P, P], F32, tag="s_ps")
                nc.tensor.matmul(ps[:rows, :rows],
                                 qk_t[hp:hp + D, 6 + hc, st * P:st * P + rows],
                                 qk_t[hp:hp + D, hc, st * P:st * P + rows],
                                 start=True, stop=True)
                pe = pp.tile([P, P], BF, tag="p_e")
                nc.scalar.activation(out=pe[:rows, :rows], in_=ps[:rows, :rows],
                                     func=AF.Exp, scale=SCALE)
                pt = pp.tile([P, P], BF, tag="p_t")
                nc.vector.tensor_mul(out=pt[:rows, :rows], in0=pe[:rows, :rows],
                                     in1=m01[:rows, :rows])
                po = op.tile([D + 1, P], F32, tag="o_ps")
                nc.tensor.matmul(po[:, :rows], v_t[0:rows, st, h, 0:D + 1],
                                 pt[:rows, :rows], start=True, stop=True)
                nc.vector.tensor_copy(out=oT_t[hp:hp + D, hc, st * P:st * P + rows],
                                      in_=po[0:D, :rows])
                # compute engines can't write partition base h; stage + DMA
                sgst = pp.tile([1, P], F32, tag="sg_st")
                nc.vector.tensor_copy(out=sgst[:, :rows], in_=po[D:D + 1, :rows])
                nc.sync.dma_start(sigma[h:h + 1, st * P:st * P + rows],
                                  sgst[:, :rows])
        with nc.allow_low_precision(reason="rinv bf16 feeds bcast matmul"):
            nc.vector.reciprocal(out=rinv, in_=sigma)
        for pr in range(CO):
            for b in range(_ceil(NT, 512)):
                cols = min(512, NT - b * 512)
                bc = bcp.tile([P, 512], F32, tag="bc_ps")
                nc.tensor.matmul(bc[:, :cols], sel_sb[:, pr * P:(pr + 1) * P],
                                 rinv[:, b * 512:b * 512 + cols],
                                 start=True, stop=True)
                nc.vector.tensor_mul(out=oT_t[:, pr, b * 512:b * 512 + cols],
                                     in0=oT_t[:, pr, b * 512:b * 512 + cols],
                                     in1=bc[:, :cols])


def phase3_temporal_proj(nc, tc, x_in, wptfc, oT_t, xs_d):
    """xt = x + oT @ (tfc_w @ tproj_w).T, scattered to spatial layout."""
    with ExitStack() as ctx:
        mp = ctx.enter_context(tc.tile_pool(name="p3_ps", bufs=4, space="PSUM"))
        tp = ctx.enter_context(tc.tile_pool(name="p3_t", bufs=4))

        # xs_d[f, 0, :] = x[0] (cls) for every frame
        cls_sb = tp.tile([8, C], BF, tag="cls_sb")
        nc.gpsimd.dma_start(cls_sb, bass.AP(tensor=x_in, offset=0,
                                            ap=[[0, 8], [1, C]]))
        nc.sync.dma_start(xs_d[:, 0, :], cls_sb)

        for i in range(_ceil(NT, P)):
            tok0 = i * P
            rows = min(P, NT - tok0)
            x_t = tp.tile([P, C], BF, tag="x_t3")
            nc.sync.dma_start(x_t[:rows], x_in[1 + tok0:1 + tok0 + rows, :])
            xt = tp.tile([P, C], BF, tag="xt3")
            for half, fcols in ((0, 512), (1, 256)):
                ps = mp.tile([P, 512], F32, tag="p3ps")
                for co in range(CO):
                    nc.tensor.matmul(
                        ps[:rows, :fcols], oT_t[:, co, tok0:tok0 + rows],
                        wptfc[:, co, half * 512:half * 512 + fcols],
                        start=(co == 0), stop=(co == CO - 1))
                nc.vector.tensor_add(
                    out=xt[:rows, half * 512:half * 512 + fcols],
                    in0=ps[:rows, :fcols],
                    in1=x_t[:rows, half * 512:half * 512 + fcols])
            # token g*8+t -> xs_d[t, 1+g]; permutation on the DRAM-side AP
            g0 = tok0 // T
            ng = rows // T
            nc.sync.dma_start(
                xs_d[:, 1 + g0:1 + g0 + ng, :].rearrange("t g c -> g t c"),
                xt[:rows])


def phase5_spatial_attn(nc, tc, qk_s, v_s, sel_sb, oT_s):
    with ExitStack() as ctx:
        sp = ctx.enter_context(tc.tile_pool(name="s_sps", bufs=2, space="PSUM"))
        op = ctx.enter_context(tc.tile_pool(name="s_ops", bufs=3, space="PSUM"))
        pp = ctx.enter_context(tc.tile_pool(name="s_p", bufs=4))
        sig = ctx.enter_context(tc.tile_pool(name="s_sig", bufs=2))
        bcp = ctx.enter_context(tc.tile_pool(name="s_bc", bufs=2, space="PSUM"))

        for f in range(8):
            col0 = f * NSEQ
            sigma = sig.tile([12, NSEQ], F32, tag="sig_s")
            rinv = sig.tile([12, NSEQ], BF, tag="rinv_s")
            for h in range(H):
                hp, hc = (h % 2) * D, h // 2
                po = op.tile([D + 1, NSEQ], F32, tag="o_ps_s")
                for ch, off, chlen in ((0, 0, P), (1, P, NSEQ - P)):
                    ps = sp.tile([P, NSEQ], F32, tag="s_ps_s")
                    nc.tensor.matmul(
                        ps[:chlen],
                        qk_s[hp:hp + D, 6 + hc, col0 + off:col0 + off + chlen],
                        qk_s[hp:hp + D, hc, col0:col0 + NSEQ],
                        start=True, stop=True)
                    pt = pp.tile([P, NSEQ], BF, tag="p_s")
                    nc.scalar.activation(out=pt[:chlen], in_=ps[:chlen],
                                         func=AF.Exp, scale=SCALE)
                    nc.tensor.matmul(po, v_s[0:chlen, f, ch, h, 0:D + 1],
                                     pt[:chlen], start=(ch == 0), stop=(ch == 1))
                nc.vector.tensor_copy(out=oT_s[hp:hp + D, hc, col0:col0 + NSEQ],
                                      in_=po[0:D])
                sgst = pp.tile([1, NSEQ], F32, tag="sg_st")
                nc.vector.tensor_copy(out=sgst, in_=po[D:D + 1])
                nc.sync.dma_start(sigma[h:h + 1], sgst)
            with nc.allow_low_precision(reason="rinv bf16 feeds bcast matmul"):
                nc.vector.reciprocal(out=rinv, in_=sigma)
            for pr in range(CO):
                bc = bcp.tile([P, NSEQ], F32, tag="bc_s")
                nc.tensor.matmul(bc, sel_sb[:, pr * P:(pr + 1) * P], rinv,
                                 start=True, stop=True)
                nc.vector.tensor_mul(out=oT_s[:, pr, col0:col0 + NSEQ],
                                     in0=oT_s[:, pr, col0:col0 + NSEQ], in1=bc)


def phase6_spatial_proj(nc, tc, xs_d, wproj_s, oT_s, y_s):
    with ExitStack() as ctx:
        mp = ctx.enter_context(tc.tile_pool(name="p6_ps", bufs=4, space="PSUM"))
        tp = ctx.enter_context(tc.tile_pool(name="p6_t", bufs=4))
        xs_flat = xs_d[:].rearrange("f n c -> (f n) c")
        ys_flat = y_s[:].rearrange("f n c -> (f n) c")
        for i in range(_ceil(NS, P)):
            rows = min(P, NS - i * P)
            x_t = tp.tile([P, C], BF, tag="x_t6")
            nc.sync.dma_start(x_t[:rows], xs_flat[i * P:i * P + rows, :])
            yt = tp.tile([P, C], BF, tag="yt6")
            for half, fcols in ((0, 512), (1, 256)):
                ps = mp.tile([P, 512], F32, tag="p6ps")
                for co in range(CO):
                    nc.tensor.matmul(
                        ps[:rows, :fcols], oT_s[:, co, i * P:i * P + rows],
                        wproj_s[:, co, half * 512:half * 512 + fcols],
                        start=(co == 0), stop=(co == CO - 1))
                nc.vector.tensor_add(
                    out=yt[:rows, half * 512:half * 512 + fcols],
                    in0=ps[:rows, :fcols],
                    in1=x_t[:rows, half * 512:half * 512 + fcols])
            nc.sync.dma_start(ys_flat[i * P:i * P + rows, :], yt[:rows])


def phase7_cls(nc, tc, y_s, one8_in, cls_row):
    with ExitStack() as ctx:
        tp = ctx.enter_context(tc.tile_pool(name="p7_t", bufs=1))
        mp = ctx.enter_context(tc.tile_pool(name="p7_ps", bufs=2, space="PSUM"))
        ycls = tp.tile([8, C], BF)
        nc.sync.dma_start(ycls, y_s[:, 0, :])
        o8 = tp.tile([8, 1], BF)
        nc.sync.dma_start(o8, one8_in[:])
        res = tp.tile([1, C], BF)
        for half, fcols in ((0, 512), (1, 256)):
            ps = mp.tile([1, 512], F32, tag="p7_ps")
            nc.tensor.matmul(ps[:, :fcols], o8,
                             ycls[:, half * 512:half * 512 + fcols],
                             start=True, stop=True)
            nc.vector.tensor_copy(out=res[:, half * 512:half * 512 + fcols],
                                  in_=ps[:, :fcols])
        nc.sync.dma_start(cls_row[:], res)


def phase8_mlp(nc, tc, y_s, cls_row, wfc1, wfc2, out, ident, eps_t):
    blocks = [(0, 512), (512, 512), (1024, 512), (1536, 33)]
    with ExitStack() as ctx:
        lnp = ctx.enter_context(tc.tile_pool(name="p8_ln", bufs=3))
        lnout = ctx.enter_context(tc.tile_pool(name="p8_lno", bufs=3))
        xrp = ctx.enter_context(tc.tile_pool(name="p8_xr", bufs=5))
        tpp = ctx.enter_context(tc.tile_pool(name="p8_tp", bufs=3, space="PSUM"))
        xlp = ctx.enter_context(tc.tile_pool(name="p8_xlT", bufs=2))
        mp = ctx.enter_context(tc.tile_pool(name="p8_ps", bufs=4, space="PSUM"))
        h1p = ctx.enter_context(tc.tile_pool(name="p8_h1", bufs=2))
        otp = ctx.enter_context(tc.tile_pool(name="p8_o", bufs=3))

        for tok0, btok in blocks:
            nits = _ceil(btok, P)
            xlnT = xlp.tile([P, CO, 512], BF, tag="xlnT8")
            xts = []
            for it in range(nits):
                rows = min(P, btok - it * P)
                x_t = xrp.tile([P, C], BF, tag="x_t8")
                g0 = (tok0 + it * P) // T
                if btok == 33:
                    nc.sync.dma_start(
                        x_t[:32],
                        y_s[:, 1 + g0:1 + g0 + 4, :].rearrange("t g c -> g t c"))
                    nc.sync.dma_start(x_t[32:33], cls_row[:])
                else:
                    nc.sync.dma_start(
                        x_t[:rows],
                        y_s[:, 1 + g0:1 + g0 + 16, :].rearrange("t g c -> g t c"))
                xts.append((x_t, rows))
                xln = ln_tile(nc, lnp, lnout, x_t, rows, eps_t)
                for co in range(CO):
                    pt = tpp.tile([P, P], BF, tag="tp8")
                    nc.tensor.transpose(pt[:, :rows],
                                        xln[:rows, co * P:(co + 1) * P],
                                        ident[:rows, :rows])
                    nc.vector.tensor_copy(out=xlnT[:, co, it * P:it * P + rows],
                                          in_=pt[:, :rows])
            h1 = h1p.tile([P, HIDO, 512], BF, tag="h1blk")
            for o in range(HIDO):
                ps = mp.tile([P, 512], F32, tag="p8ps")
                for co in range(CO):
                    nc.tensor.matmul(ps[:, :btok], wfc1[:, co, o * P:(o + 1) * P],
                                     xlnT[:, co, :btok],
                                     start=(co == 0), stop=(co == CO - 1))
                nc.scalar.activation(out=h1[:, o, :btok], in_=ps[:, :btok],
                                     func=AF.Gelu)
            for it in range(nits):
                x_t, rows = xts[it]
                o_t = otp.tile([P, C], F32, tag="o_t8")
                for half, fcols in ((0, 512), (1, 256)):
                    ps = mp.tile([P, 512], F32, tag="p8ps")
                    for ho in range(HIDO):
                        nc.tensor.matmul(
                            ps[:rows, :fcols], h1[:, ho, it * P:it * P + rows],
                            wfc2[:, ho, half * 512:half * 512 + fcols],
                            start=(ho == 0), stop=(ho == HIDO - 1))
                    nc.vector.tensor_add(
                        out=o_t[:rows, half * 512:half * 512 + fcols],
                        in0=ps[:rows, :fcols],
                        in1=x_t[:rows, half * 512:half * 512 + fcols])
                row0 = tok0 + it * P
                if btok == 33:
                    nc.sync.dma_start(out[1 + row0:1 + row0 + 32, :], o_t[:32])
                    nc.sync.dma_start(out[0:1, :], o_t[32:33])
                else:
                    nc.sync.dma_start(out[1 + row0:1 + row0 + rows, :],
                                      o_t[:rows])


def build_nc():
    nc = bacc.Bacc("TRN2", target_bir_lowering=False, debug=False)

    x_in = nc.dram_tensor("x", (N, C), BF, kind="ExternalInput")
    w_qk_t = nc.dram_tensor("w_qk_t", (C, 2 * C), BF, kind="ExternalInput")
    w_v_t = nc.dram_tensor("w_v_t", (C, C), BF, kind="ExternalInput")
    w_qk_s = nc.dram_tensor("w_qk_s", (C, 2 * C), BF, kind="ExternalInput")
    w_v_s = nc.dram_tensor("w_v_s", (C, C), BF, kind="ExternalInput")
    w_ptfc = nc.dram_tensor("w_ptfc", (C, C), BF, kind="ExternalInput")
    w_proj_s = nc.dram_tensor("w_proj_s", (C, C), BF, kind="ExternalInput")
    w_fc1 = nc.dram_tensor("w_fc1", (C, HID), BF, kind="ExternalInput")
    w_fc2 = nc.dram_tensor("w_fc2", (HID, C), BF, kind="ExternalInput")
    sel12 = nc.dram_tensor("sel12", (12, C), BF, kind="ExternalInput")
    one8 = nc.dram_tensor("one8", (8, 1), BF, kind="ExternalInput")
    m01_in = nc.dram_tensor("m01", (P, P), BF, kind="ExternalInput")
    ident_in = nc.dram_tensor("ident", (P, P), BF, kind="ExternalInput")
    out = nc.dram_tensor("out", (N, C), F32, kind="ExternalOutput")
    dbg = {}
    if KDEBUG:
        for nm, shp in (("d_qk_t", (P, 12, NT)), ("d_v_t", (P, 13, 12, D + 1)),
                        ("d_oT_t", (P, CO, NT)), ("d_xs", (8, NSEQ, C)),
                        ("d_qk_s", (P, 12, NS)),
                        ("d_v_s", (P, 8, 2, 12, D + 1)),
                        ("d_oT_s", (P, CO, NS)), ("d_y_s", (8, NSEQ, C)),
                        ("d_cls", (1, C))):
            dbg[nm] = nc.dram_tensor(nm, shp, BF, kind="ExternalOutput")

    with tile.TileContext(nc) as tc:
        with ExitStack() as root:
            dram = root.enter_context(tc.tile_pool(name="dram", bufs=1,
                                                   space="DRAM"))
            const = root.enter_context(tc.tile_pool(name="const", bufs=1))
            xs_d = dram.tile([8, NSEQ, C], BF)
            y_s = dram.tile([8, NSEQ, C], BF)
            cls_row = dram.tile([1, C], BF)

            ident = const.tile([P, P], BF)
            nc.gpsimd.dma_start(ident, ident_in[:])
            m01 = const.tile([P, P], BF)
            nc.gpsimd.dma_start(m01, m01_in[:])
            eps_t = const.tile([P, 1], F32)
            nc.vector.memset(eps_t, EPS)
            sel_sb = const.tile([12, C], BF)
            nc.gpsimd.dma_start(sel_sb, sel12[:])

            # Stack allocator: a pool reserves its full size at open, so
            # lifetimes must nest (LIFO).  Onion order, outermost first;
            # tiles/DMAs are issued later at the prefetch point.
            sWL = ExitStack()   # wfc1                [t0 .. end]
            wL = sWL.enter_context(tc.tile_pool(name="wL", bufs=1))
            sWC = ExitStack()   # wqk_s, wv_s         [t0 .. ph6]
            wC = sWC.enter_context(tc.tile_pool(name="wC", bufs=1))
            sWD = ExitStack()   # wproj_s             [t0 .. ph6]
            wD = sWD.enter_context(tc.tile_pool(name="wD", bufs=1))

            # ---- temporal phases 1-3 ----
            sPT = ExitStack()   # qk_t, v_t           [ph1 .. ph3]
            poolT = sPT.enter_context(tc.tile_pool(name="poolT", bufs=1))
            qk_t = poolT.tile([P, 12, NT], BF)
            v_t = poolT.tile([P, 13, 12, D + 1], BF)
            nc.gpsimd.memset(v_t[:, :, :, D:D + 1], 1.0)

            sWA = ExitStack()   # wqk_t, wv_t         [ph1]
            wA = sWA.enter_context(tc.tile_pool(name="wA", bufs=1))
            wqk_t = wA.tile([P, CO, 2 * C], BF)
            nc.gpsimd.dma_start(wqk_t, w_qk_t[:].rearrange("(co p) o -> p co o", p=P))
            wv_t = wA.tile([P, CO, C], BF)
            nc.gpsimd.dma_start(wv_t, w_v_t[:].rearrange("(co p) o -> p co o", p=P))

            if "1" in PHASES:
                with ExitStack() as ctx:
                    def src_t(x_t, i, rows):
                        nc.sync.dma_start(x_t[:rows],
                                          x_in[1 + i * P:1 + i * P + rows, :])
                    vchunks = [
                        ((lambda st: (lambda chlen, half:
                            v_t[0:chlen, st, half * 8:half * 8 + (8, 4)[half],
                                0:D]))(st),
                         st * P, min(P, NT - st * P))
                        for st in range(13)]
                    qkv_phase(nc, tc, ctx, src_t, NT, wqk_t, wv_t,
                              qk_t, v_t, vchunks, ident, eps_t)
            sWA.close()

            sWB = ExitStack()   # wproj_t, wtfc       [ph2 .. ph3]
            wB = sWB.enter_context(tc.tile_pool(name="wB", bufs=1))
            wptfc = wB.tile([P, CO, C], BF)
            nc.gpsimd.dma_start(wptfc,
                                w_ptfc[:].rearrange("(co p) o -> p co o", p=P))
            sOT = ExitStack()   # oT_t                [ph2 .. ph3]
            otp_ = sOT.enter_context(tc.tile_pool(name="oT_t_pool", bufs=1))
            oT_t = otp_.tile([P, CO, NT], BF)

            # prefetch phase-4 weights during phases 2-3
            wqk_s = wC.tile([P, CO, 2 * C], BF)
            nc.gpsimd.dma_start(wqk_s, w_qk_s[:].rearrange("(co p) o -> p co o", p=P))
            wv_s = wC.tile([P, CO, C], BF)
            nc.gpsimd.dma_start(wv_s, w_v_s[:].rearrange("(co p) o -> p co o", p=P))

            if "2" in PHASES:
                phase2_temporal_attn(nc, tc, qk_t, v_t, m01, sel_sb, oT_t)
                if KDEBUG:
                    nc.sync.dma_start(dbg["d_qk_t"][:], qk_t)
                    nc.sync.dma_start(dbg["d_v_t"][:], v_t)
                    nc.sync.dma_start(dbg["d_oT_t"][:], oT_t)
            if "3" in PHASES:
                phase3_temporal_proj(nc, tc, x_in, wptfc, oT_t, xs_d)
            sOT.close()
            sWB.close()
            sPT.close()

            # ---- spatial phases 4-6 ----
            wproj_s = wD.tile([P, CO, C], BF)
            nc.gpsimd.dma_start(wproj_s,
                                w_proj_s[:].rearrange("(co p) o -> p co o", p=P))
            wfc1 = wL.tile([P, CO, HID], BF)
            nc.gpsimd.dma_start(wfc1, w_fc1[:].rearrange("(co p) o -> p co o", p=P))

            sPS = ExitStack()   # qk_s, v_s, oT_s     [ph4 .. ph6]
            poolS = sPS.enter_context(tc.tile_pool(name="poolS", bufs=1))
            qk_s = poolS.tile([P, 12, NS], BF)
            v_s = poolS.tile([P, 8, 2, 12, D + 1], BF)
            nc.gpsimd.memset(v_s[:, :, :, :, D:D + 1], 1.0)
            oT_s = poolS.tile([P, CO, NS], BF)

            if "4" in PHASES:
                with ExitStack() as ctx:
                    xs_flat = xs_d[:].rearrange("f n c -> (f n) c")

                    def src_s(x_t, i, rows):
                        nc.sync.dma_start(x_t[:rows],
                                          xs_flat[i * P:i * P + rows, :])
                    vchunks = [
                        ((lambda f, chi: (lambda chlen, half:
                            v_s[0:chlen, f, chi, half * 8:half * 8 + (8, 4)[half],
                                0:D]))(f, chi),
                         f * NSEQ + chi * P, (P, NSEQ - P)[chi])
                        for f in range(8) for chi in range(2)]
                    qkv_phase(nc, tc, ctx, src_s, NS, wqk_s, wv_s,
                              qk_s, v_s, vchunks, ident, eps_t)
            if "5" in PHASES:
                phase5_spatial_attn(nc, tc, qk_s, v_s, sel_sb, oT_s)
                if KDEBUG:
                    nc.sync.dma_start(dbg["d_qk_s"][:], qk_s)
                    nc.sync.dma_start(dbg["d_v_s"][:], v_s)
                    nc.sync.dma_start(dbg["d_oT_s"][:], oT_s)
            if "6" in PHASES:
                phase6_spatial_proj(nc, tc, xs_d, wproj_s, oT_s, y_s)
            sPS.close()
            sWD.close()
            sWC.close()

            # ---- cls + MLP ----
            sWF = ExitStack()   # wfc2                [ph7 .. end]
            wF = sWF.enter_context(tc.tile_pool(name="wF", bufs=1))
            wfc2 = wF.tile([P, HIDO, C], BF)
            nc.gpsimd.dma_start(wfc2, w_fc2[:].rearrange("(ho p) o -> p ho o", p=P))

            if "7" in PHASES:
                phase7_cls(nc, tc, y_s, one8, cls_row)
            if "8" in PHASES:
                phase8_mlp(nc, tc, y_s, cls_row, wfc1, wfc2, out, ident, eps_t)
            sWF.close()
            sWL.close()

            if KDEBUG:
                nc.sync.dma_start(dbg["d_xs"][:], xs_d[:])
                nc.sync.dma_start(dbg["d_y_s"][:], y_s[:])
                nc.sync.dma_start(dbg["d_cls"][:], cls_row[:])

    nc.compile()
    return nc


_NC_CACHE = None


def _get_nc():
    global _NC_CACHE
    if _NC_CACHE is None:
        _NC_CACHE = build_nc()
    return _NC_CACHE


def make_consts():
    bf = ml_dtypes.bfloat16
    sel = np.zeros((12, C), np.float32)
    for pr in range(6):
        for p in range(P):
            sel[2 * pr + p // D, pr * P + p] = 1.0
    one8 = np.full((8, 1), 0.125, np.float32)
    m01 = np.zeros((P, P), np.float32)
    for s in range(16):
        m01[s * 8:(s + 1) * 8, s * 8:(s + 1) * 8] = 1.0
    ident = np.eye(P, dtype=np.float32)
    return sel.astype(bf), one8.astype(bf), m01.astype(bf), ident.astype(bf)


def kernel(**inputs):
    bf = ml_dtypes.bfloat16
    x = np.asarray(inputs["x"], dtype=np.float32)
    B = x.shape[0]
    tr = lambda w: np.ascontiguousarray(
        np.asarray(w, np.float32).T.astype(bf))
    qkv_w = np.asarray(inputs["qkv_w"], np.float32)
    tqkv_w = np.asarray(inputs["tqkv_w"], np.float32)
    sel, one8, m01, ident = make_consts()
    shared = {
        "w_qk_t": tr(tqkv_w[:2 * C]), "w_v_t": tr(tqkv_w[2 * C:]),
        "w_qk_s": tr(qkv_w[:2 * C]), "w_v_s": tr(qkv_w[2 * C:]),
        "w_ptfc": tr(np.asarray(inputs["tfc_w"], np.float32)
                     @ np.asarray(inputs["tproj_w"], np.float32)),
        "w_proj_s": tr(inputs["proj_w"]),
        "w_fc1": tr(inputs["fc1_w"]), "w_fc2": tr(inputs["fc2_w"]),
        "sel12": sel, "one8": one8, "m01": m01, "ident": ident,
    }
    nc = _get_nc()
    in_maps = [dict(shared, x=np.ascontiguousarray(x[b].astype(bf)))
               for b in range(B)]
    res = run_bass_kernel_spmd(nc, in_maps, core_ids=list(range(B)),
                               trace=bool(int(os.environ.get("KTRACE", "0"))))
    out = np.stack([res.results[b]["out"] for b in range(B)], axis=0)
    kernel.last_results = res
    return out


# revision 15
# speedup vs baseline: 1.6635x; 1.0577x over previous
"""TimeSformer-style block (temporal attn -> spatial attn -> MLP) on 8 trn2 cores.

Data-parallel over B=8: each NeuronCore processes one batch element end to end.
All GEMMs run in bf16 (1 cycle/row on the PE at any free-dim size, 4x cheaper
weight loads than fp32r); PSUM accumulation stays fp32. Attention q/k/v images
live in SBUF (no HBM round trips); the MLP is fused per 512-token block so the
fc1 activation image never touches DRAM. Block-diagonal attention masking uses
a multiplicative 0/1 mask after exp instead of extra contraction rows.
"""

import os
import sys
from contextlib import ExitStack

sys.path.insert(0, "/opt/trn_rl_repo")

import numpy as np
import ml_dtypes

import concourse.bass as bass
import concourse.mybir as mybir
import concourse.tile as tile
from concourse import bacc
from concourse.bass_utils import run_bass_kernel_spmd

F32 = mybir.dt.float32
BF = mybir.dt.bfloat16
AF = mybir.ActivationFunctionType
ALU = mybir.AluOpType

C = 768
CO = 6           # C / 128
H = 12
D = 64
T = 8
G = 196          # h*w sequences
NT = G * T       # 1568 temporal tokens
NSEQ = 197       # spatial seq len (cls + 196)
NS = 8 * NSEQ    # 1576 spatial tokens
N = 1569
HID = 3072
HIDO = 24        # HID / 128
P = 128
EPS = 1e-5
SCALE = D ** -0.5

PHASES = os.environ.get("KPHASES", "12345678")
KDEBUG = bool(int(os.environ.get("KDEBUG", "0")))


def _ceil(a, b):
    return (a + b - 1) // b


def ln_tile(nc, tmp, out_pool, x_t, rows, eps_t):
    """LayerNorm over free dim (768) of a [rows<=128, 768] token-major tile."""
    stats = tmp.tile([P, 3, 6], F32, tag="ln_stats")
    for s in range(3):
        nc.vector.bn_stats(out=stats[:rows, s], in_=x_t[:rows, s * 256:(s + 1) * 256])
    mv = tmp.tile([P, 2], F32, tag="ln_mv")
    nc.vector.bn_aggr(out=mv[:rows], in_=stats[:rows])
    nc.scalar.activation(out=mv[:rows, 1:2], in_=mv[:rows, 1:2], func=AF.Sqrt,
                         bias=eps_t[:rows], scale=1.0)
    nc.vector.reciprocal(out=mv[:rows, 1:2], in_=mv[:rows, 1:2])
    xln = out_pool.tile([P, C], BF, tag="ln_out")
    nc.vector.tensor_scalar(out=xln[:rows], in0=x_t[:rows],
                            scalar1=mv[:rows, 0:1], scalar2=mv[:rows, 1:2],
                            op0=ALU.subtract, op1=ALU.mult)
    return xln


def qkv_phase(nc, tc, ctx, src_dma, ntok, wqk, wv, qk_img, v_img, vchunks,
              ident, eps_t):
    """LN -> PE transpose -> qk (c-major SBUF image) + v (token-major chunks)."""
    lnp = ctx.enter_context(tc.tile_pool(name="lnp", bufs=3))
    lnout = ctx.enter_context(tc.tile_pool(name="lnout", bufs=3))
    tpp = ctx.enter_context(tc.tile_pool(name="tp_ps", bufs=3, space="PSUM"))
    xlp = ctx.enter_context(tc.tile_pool(name="xlnT", bufs=1))
    mmp = ctx.enter_context(tc.tile_pool(name="mm_ps", bufs=4, space="PSUM"))

    xlnT = xlp.tile([P, CO, ntok], BF)
    ntiles = _ceil(ntok, P)
    for i in range(ntiles):
        rows = min(P, ntok - i * P)
        x_t = lnp.tile([P, C], BF, tag="x_t")
        src_dma(x_t, i, rows)
        xln = ln_tile(nc, lnp, lnout, x_t, rows, eps_t)
        for cp in range(CO // 2):
            pt = tpp.tile([P, 2, P], BF, tag="tp")
            for k in range(2):
                co = 2 * cp + k
                nc.tensor.transpose(pt[:, k, :rows],
                                    xln[:rows, co * P:(co + 1) * P],
                                    ident[:rows, :rows])
            nc.vector.tensor_copy(
                out=xlnT[:, 2 * cp:2 * cp + 2, i * P:i * P + rows],
                in_=pt[:, :, :rows])
    for b in range(_ceil(ntok, 512)):
        cols = min(512, ntok - b * 512)
        for o in range(12):  # 2C/128 output chunks (q then k)
            ps = mmp.tile([P, 512], F32, tag="mm")
            for co in range(CO):
                nc.tensor.matmul(ps[:, :cols], wqk[:, co, o * P:(o + 1) * P],
                                 xlnT[:, co, b * 512:b * 512 + cols],
                                 start=(co == 0), stop=(co == CO - 1))
            nc.scalar.copy(out=qk_img[:, o, b * 512:b * 512 + cols],
                           in_=ps[:, :cols])
    for dst_fn, col0v, chlen in vchunks:
        for half, fcols in ((0, 512), (1, 256)):
            ps = mmp.tile([P, 512], F32, tag="mm")
            for co in range(CO):
                nc.tensor.matmul(ps[:chlen, :fcols],
                                 xlnT[:, co, col0v:col0v + chlen],
                                 wv[:, co, half * 512:half * 512 + fcols],
                                 start=(co == 0), stop=(co == CO - 1))
            nc.vector.tensor_copy(out=dst_fn(chlen, half), in_=ps[:chlen, :fcols])


def phase2_temporal_attn(nc, tc, qk_t, v_t, m01, sel_sb, oT_t):
    with ExitStack() as ctx:
        sp = ctx.enter_context(tc.tile_pool(name="t_sps", bufs=2, space="PSUM"))
        op = ctx.enter_context(tc.tile_pool(name="t_ops", bufs=3, space="PSUM"))
        pp = ctx.enter_context(tc.tile_pool(name="t_p", bufs=6))
        sig = ctx.enter_context(tc.tile_pool(name="t_sig", bufs=1))
        bcp = ctx.enter_context(tc.tile_pool(name="t_bc", bufs=2, space="PSUM"))

        sigma = sig.tile([12, NT], F32)
        rinv = sig.tile([12, NT], BF)
        nst = _ceil(NT, P)
        for h in range(H):
            hp, hc = (h % 2) * D, h // 2
            for st in range(nst):
                rows = min(P, NT - st * P)
                ps = sp.tile([P, P], F32, tag="s_ps")
                nc.tensor.matmul(ps[:rows, :rows],
                                 qk_t[hp:hp + D, 6 + hc, st * P:st * P + rows],
                                 qk_t[hp:hp + D, hc, st * P:st * P + rows],
                                 start=True, stop=True)
                pe = pp.tile([P, P], BF, tag="p_e")
                nc.scalar.activation(out=pe[:rows, :rows], in_=ps[:rows, :rows],
                                     func=AF.Exp, scale=SCALE)
                pt = pp.tile([P, P], BF, tag="p_t")
                nc.gpsimd.tensor_mul(out=pt[:rows, :rows], in0=pe[:rows, :rows],
                                     in1=m01[:rows, :rows])
                po = op.tile([D + 1, P], F32, tag="o_ps")
                nc.tensor.matmul(po[:, :rows], v_t[0:rows, st, h, 0:D + 1],
                                 pt[:rows, :rows], start=True, stop=True)
                nc.vector.tensor_copy(out=oT_t[hp:hp + D, hc, st * P:st * P + rows],
                                      in_=po[0:D, :rows])
                # compute engines can't write partition base h; stage + DMA
                sgst = pp.tile([1, P], F32, tag="sg_st")
                nc.scalar.copy(out=sgst[:, :rows], in_=po[D:D + 1, :rows])
                nc.sync.dma_start(sigma[h:h + 1, st * P:st * P + rows],
                                  sgst[:, :rows])
        with nc.allow_low_precision(reason="rinv bf16 feeds bcast matmul"):
            nc.vector.reciprocal(out=rinv, in_=sigma)
        for pr in range(CO):
            for b in range(_ceil(NT, 512)):
                cols = min(512, NT - b * 512)
                bc = bcp.tile([P, 512], F32, tag="bc_ps")
                nc.tensor.matmul(bc[:, :cols], sel_sb[:, pr * P:(pr + 1) * P],
                                 rinv[:, b * 512:b * 512 + cols],
                                 start=True, stop=True)
                nc.vector.tensor_mul(out=oT_t[:, pr, b * 512:b * 512 + cols],
                                     in0=oT_t[:, pr, b * 512:b * 512 + cols],
                                     in1=bc[:, :cols])


def phase3_temporal_proj(nc, tc, x_in, wptfc, oT_t, xs_d):
    """xt = x + oT @ (tfc_w @ tproj_w).T, scattered to spatial layout."""
    with ExitStack() as ctx:
        mp = ctx.enter_context(tc.tile_pool(name="p3_ps", bufs=4, space="PSUM"))
        tp = ctx.enter_context(tc.tile_pool(name="p3_t", bufs=4))

        # xs_d[f, 0, :] = x[0] (cls) for every frame
        cls_sb = tp.tile([8, C], BF, tag="cls_sb")
        nc.gpsimd.dma_start(cls_sb, bass.AP(tensor=x_in, offset=0,
                                            ap=[[0, 8], [1, C]]))
        nc.sync.dma_start(xs_d[:, 0, :], cls_sb)

        for i in range(_ceil(NT, P)):
            tok0 = i * P
            rows = min(P, NT - tok0)
            x_t = tp.tile([P, C], BF, tag="x_t3")
            nc.sync.dma_start(x_t[:rows], x_in[1 + tok0:1 + tok0 + rows, :])
            xt = tp.tile([P, C], BF, tag="xt3")
            for half, fcols in ((0, 512), (1, 256)):
                ps = mp.tile([P, 512], F32, tag="p3ps")
                for co in range(CO):
                    nc.tensor.matmul(
                        ps[:rows, :fcols], oT_t[:, co, tok0:tok0 + rows],
                        wptfc[:, co, half * 512:half * 512 + fcols],
                        start=(co == 0), stop=(co == CO - 1))
                nc.vector.tensor_add(
                    out=xt[:rows, half * 512:half * 512 + fcols],
                    in0=ps[:rows, :fcols],
                    in1=x_t[:rows, half * 512:half * 512 + fcols])
            # token g*8+t -> xs_d[t, 1+g]; permutation on the DRAM-side AP
            g0 = tok0 // T
            ng = rows // T
            nc.sync.dma_start(
                xs_d[:, 1 + g0:1 + g0 + ng, :].rearrange("t g c -> g t c"),
                xt[:rows])


def phase5_spatial_attn(nc, tc, qk_s, v_s, sel_sb, oT_s):
    with ExitStack() as ctx:
        sp = ctx.enter_context(tc.tile_pool(name="s_sps", bufs=2, space="PSUM"))
        op = ctx.enter_context(tc.tile_pool(name="s_ops", bufs=3, space="PSUM"))
        pp = ctx.enter_context(tc.tile_pool(name="s_p", bufs=4))
        sig = ctx.enter_context(tc.tile_pool(name="s_sig", bufs=2))
        bcp = ctx.enter_context(tc.tile_pool(name="s_bc", bufs=2, space="PSUM"))

        for f in range(8):
            col0 = f * NSEQ
            sigma = sig.tile([12, NSEQ], F32, tag="sig_s")
            rinv = sig.tile([12, NSEQ], BF, tag="rinv_s")
            for h in range(H):
                hp, hc = (h % 2) * D, h // 2
                po = op.tile([D + 1, NSEQ], F32, tag="o_ps_s")
                for ch, off, chlen in ((0, 0, P), (1, P, NSEQ - P)):
                    ps = sp.tile([P, NSEQ], F32, tag="s_ps_s")
                    nc.tensor.matmul(
                        ps[:chlen],
                        qk_s[hp:hp + D, 6 + hc, col0 + off:col0 + off + chlen],
                        qk_s[hp:hp + D, hc, col0:col0 + NSEQ],
                        start=True, stop=True)
                    pt = pp.tile([P, NSEQ], BF, tag="p_s")
                    nc.scalar.activation(out=pt[:chlen], in_=ps[:chlen],
                                         func=AF.Exp, scale=SCALE)
                    nc.tensor.matmul(po, v_s[0:chlen, f, ch, h, 0:D + 1],
                                     pt[:chlen], start=(ch == 0), stop=(ch == 1))
                nc.vector.tensor_copy(out=oT_s[hp:hp + D, hc, col0:col0 + NSEQ],
                                      in_=po[0:D])
                sgst = pp.tile([1, NSEQ], F32, tag="sg_st")
                nc.scalar.copy(out=sgst, in_=po[D:D + 1])
                nc.sync.dma_start(sigma[h:h + 1], sgst)
            with nc.allow_low_precision(reason="rinv bf16 feeds bcast matmul"):
                nc.vector.reciprocal(out=rinv, in_=sigma)
            for pr in range(CO):
                bc = bcp.tile([P, NSEQ], F32, tag="bc_s")
                nc.tensor.matmul(bc, sel_sb[:, pr * P:(pr + 1) * P], rinv,
                                 start=True, stop=True)
                nc.vector.tensor_mul(out=oT_s[:, pr, col0:col0 + NSEQ],
                                     in0=oT_s[:, pr, col0:col0 + NSEQ], in1=bc)


def phase6_spatial_proj(nc, tc, xs_d, wproj_s, oT_s, y_s):
    with ExitStack() as ctx:
        mp = ctx.enter_context(tc.tile_pool(name="p6_ps", bufs=4, space="PSUM"))
        tp = ctx.enter_context(tc.tile_pool(name="p6_t", bufs=4))
        xs_flat = xs_d[:].rearrange("f n c -> (f n) c")
        ys_flat = y_s[:].rearrange("f n c -> (f n) c")
        for i in range(_ceil(NS, P)):
            rows = min(P, NS - i * P)
            x_t = tp.tile([P, C], BF, tag="x_t6")
            nc.sync.dma_start(x_t[:rows], xs_flat[i * P:i * P + rows, :])
            yt = tp.tile([P, C], BF, tag="yt6")
            for half, fcols in ((0, 512), (1, 256)):
                ps = mp.tile([P, 512], F32, tag="p6ps")
                for co in range(CO):
                    nc.tensor.matmul(
                        ps[:rows, :fcols], oT_s[:, co, i * P:i * P + rows],
                        wproj_s[:, co, half * 512:half * 512 + fcols],
                        start=(co == 0), stop=(co == CO - 1))
                nc.vector.tensor_add(
                    out=yt[:rows, half * 512:half * 512 + fcols],
                    in0=ps[:rows, :fcols],
                    in1=x_t[:rows, half * 512:half * 512 + fcols])
            nc.sync.dma_start(ys_flat[i * P:i * P + rows, :], yt[:rows])


def phase7_cls(nc, tc, y_s, one8_in, cls_row):
    with ExitStack() as ctx:
        tp = ctx.enter_context(tc.tile_pool(name="p7_t", bufs=1))
        mp = ctx.enter_context(tc.tile_pool(name="p7_ps", bufs=2, space="PSUM"))
        ycls = tp.tile([8, C], BF)
        nc.sync.dma_start(ycls, y_s[:, 0, :])
        o8 = tp.tile([8, 1], BF)
        nc.sync.dma_start(o8, one8_in[:])
        res = tp.tile([1, C], BF)
        for half, fcols in ((0, 512), (1, 256)):
            ps = mp.tile([1, 512], F32, tag="p7_ps")
            nc.tensor.matmul(ps[:, :fcols], o8,
                             ycls[:, half * 512:half * 512 + fcols],
                             start=True, stop=True)
            nc.vector.tensor_copy(out=res[:, half * 512:half * 512 + fcols],
                                  in_=ps[:, :fcols])
        nc.sync.dma_start(cls_row[:], res)


def phase8_mlp(nc, tc, y_s, cls_row, wfc1, wfc2, out, ident, eps_t):
    blocks = [(0, 512), (512, 512), (1024, 512), (1536, 33)]
    with ExitStack() as ctx:
        lnp = ctx.enter_context(tc.tile_pool(name="p8_ln", bufs=3))
        lnout = ctx.enter_context(tc.tile_pool(name="p8_lno", bufs=3))
        xrp = ctx.enter_context(tc.tile_pool(name="p8_xr", bufs=5))
        tpp = ctx.enter_context(tc.tile_pool(name="p8_tp", bufs=3, space="PSUM"))
        xlp = ctx.enter_context(tc.tile_pool(name="p8_xlT", bufs=2))
        mp = ctx.enter_context(tc.tile_pool(name="p8_ps", bufs=4, space="PSUM"))
        h1p = ctx.enter_context(tc.tile_pool(name="p8_h1", bufs=2))
        otp = ctx.enter_context(tc.tile_pool(name="p8_o", bufs=3))

        for tok0, btok in blocks:
            nits = _ceil(btok, P)
            xlnT = xlp.tile([P, CO, 512], BF, tag="xlnT8")
            xts = []
            for it in range(nits):
                rows = min(P, btok - it * P)
                x_t = xrp.tile([P, C], BF, tag="x_t8")
                g0 = (tok0 + it * P) // T
                if btok == 33:
                    nc.sync.dma_start(
                        x_t[:32],
                        y_s[:, 1 + g0:1 + g0 + 4, :].rearrange("t g c -> g t c"))
                    nc.sync.dma_start(x_t[32:33], cls_row[:])
                else:
                    nc.sync.dma_start(
                        x_t[:rows],
                        y_s[:, 1 + g0:1 + g0 + 16, :].rearrange("t g c -> g t c"))
                xts.append((x_t, rows))
                xln = ln_tile(nc, lnp, lnout, x_t, rows, eps_t)
                for cp in range(CO // 2):
                    pt = tpp.tile([P, 2, P], BF, tag="tp8")
                    for k in range(2):
                        co = 2 * cp + k
                        nc.tensor.transpose(pt[:, k, :rows],
                                            xln[:rows, co * P:(co + 1) * P],
                                            ident[:rows, :rows])
                    nc.vector.tensor_copy(
                        out=xlnT[:, 2 * cp:2 * cp + 2, it * P:it * P + rows],
                        in_=pt[:, :, :rows])
            h1 = h1p.tile([P, HIDO, 512], BF, tag="h1blk")
            for o in range(HIDO):
                ps = mp.tile([P, 512], F32, tag="p8ps")
                for co in range(CO):
                    nc.tensor.matmul(ps[:, :btok], wfc1[:, co, o * P:(o + 1) * P],
                                     xlnT[:, co, :btok],
                                     start=(co == 0), stop=(co == CO - 1))
                nc.scalar.activation(out=h1[:, o, :btok], in_=ps[:, :btok],
                                     func=AF.Gelu)
            for it in range(nits):
                x_t, rows = xts[it]
                o_t = otp.tile([P, C], F32, tag="o_t8")
                for half, fcols in ((0, 512), (1, 256)):
                    ps = mp.tile([P, 512], F32, tag="p8ps")
                    for ho in range(HIDO):
                        nc.tensor.matmul(
                            ps[:rows, :fcols], h1[:, ho, it * P:it * P + rows],
                            wfc2[:, ho, half * 512:half * 512 + fcols],
                            start=(ho == 0), stop=(ho == HIDO - 1))
                    nc.vector.tensor_add(
                        out=o_t[:rows, half * 512:half * 512 + fcols],
                        in0=ps[:rows, :fcols],
                        in1=x_t[:rows, half * 512:half * 512 + fcols])
                row0 = tok0 + it * P
                if btok == 33:
                    nc.sync.dma_start(out[1 + row0:1 + row0 + 32, :], o_t[:32])
                    nc.sync.dma_start(out[0:1, :], o_t[32:33])
                else:
                    nc.sync.dma_start(out[1 + row0:1 + row0 + rows, :],
                                      o_t[:rows])


def build_nc():
    nc = bacc.Bacc("TRN2", target_bir_lowering=False, debug=False)

    x_in = nc.dram_tensor("x", (N, C), BF, kind="ExternalInput")
    w_qk_t = nc.dram_tensor("w_qk_t", (C, 2 * C), BF, kind="ExternalInput")
    w_v_t = nc.dram_tensor("w_v_t", (C, C), BF, kind="ExternalInput")
    w_qk_s = nc.dram_tensor("w_qk_s", (C, 2 * C), BF, kind="ExternalInput")
    w_v_s = nc.dram_tensor("w_v_s", (C, C), BF, kind="ExternalInput")
    w_ptfc = nc.dram_tensor("w_ptfc", (C, C), BF, kind="ExternalInput")
    w_proj_s = nc.dram_tensor("w_proj_s", (C, C), BF, kind="ExternalInput")
    w_fc1 = nc.dram_tensor("w_fc1", (C, HID), BF, kind="ExternalInput")
    w_fc2 = nc.dram_tensor("w_fc2", (HID, C), BF, kind="ExternalInput")
    sel12 = nc.dram_tensor("sel12", (12, C), BF, kind="ExternalInput")
    one8 = nc.dram_tensor("one8", (8, 1), BF, kind="ExternalInput")
    m01_in = nc.dram_tensor("m01", (P, P), BF, kind="ExternalInput")
    ident_in = nc.dram_tensor("ident", (P, P), BF, kind="ExternalInput")
    out = nc.dram_tensor("out", (N, C), F32, kind="ExternalOutput")
    dbg = {}
    if KDEBUG:
        for nm, shp in (("d_qk_t", (P, 12, NT)), ("d_v_t", (P, 13, 12, D + 1)),
                        ("d_oT_t", (P, CO, NT)), ("d_xs", (8, NSEQ, C)),
                        ("d_qk_s", (P, 12, NS)),
                        ("d_v_s", (P, 8, 2, 12, D + 1)),
                        ("d_oT_s", (P, CO, NS)), ("d_y_s", (8, NSEQ, C)),
                        ("d_cls", (1, C))):
            dbg[nm] = nc.dram_tensor(nm, shp, BF, kind="ExternalOutput")

    with tile.TileContext(nc) as tc:
        with ExitStack() as root:
            dram = root.enter_context(tc.tile_pool(name="dram", bufs=1,
                                                   space="DRAM"))
            const = root.enter_context(tc.tile_pool(name="const", bufs=1))
            xs_d = dram.tile([8, NSEQ, C], BF)
            y_s = dram.tile([8, NSEQ, C], BF)
            cls_row = dram.tile([1, C], BF)

            ident = const.tile([P, P], BF)
            nc.gpsimd.dma_start(ident, ident_in[:])
            m01 = const.tile([P, P], BF)
            nc.gpsimd.dma_start(m01, m01_in[:])
            eps_t = const.tile([P, 1], F32)
            nc.vector.memset(eps_t, EPS)
            sel_sb = const.tile([12, C], BF)
            nc.gpsimd.dma_start(sel_sb, sel12[:])

            # Stack allocator: a pool reserves its full size at open, so
            # lifetimes must nest (LIFO).  Onion order, outermost first;
            # tiles/DMAs are issued later at the prefetch point.
            sWL = ExitStack()   # wfc1                [t0 .. end]
            wL = sWL.enter_context(tc.tile_pool(name="wL", bufs=1))
            sWC = ExitStack()   # wqk_s, wv_s         [t0 .. ph6]
            wC = sWC.enter_context(tc.tile_pool(name="wC", bufs=1))
            sWD = ExitStack()   # wproj_s             [t0 .. ph6]
            wD = sWD.enter_context(tc.tile_pool(name="wD", bufs=1))

            # ---- temporal phases 1-3 ----
            sPT = ExitStack()   # qk_t, v_t           [ph1 .. ph3]
            poolT = sPT.enter_context(tc.tile_pool(name="poolT", bufs=1))
            qk_t = poolT.tile([P, 12, NT], BF)
            v_t = poolT.tile([P, 13, 12, D + 1], BF)
            nc.gpsimd.memset(v_t[:, :, :, D:D + 1], 1.0)

            sWA = ExitStack()   # wqk_t, wv_t         [ph1]
            wA = sWA.enter_context(tc.tile_pool(name="wA", bufs=1))
            wqk_t = wA.tile([P, CO, 2 * C], BF)
            nc.gpsimd.dma_start(wqk_t, w_qk_t[:].rearrange("(co p) o -> p co o", p=P))
            wv_t = wA.tile([P, CO, C], BF)
            nc.gpsimd.dma_start(wv_t, w_v_t[:].rearrange("(co p) o -> p co o", p=P))

            if "1" in PHASES:
                with ExitStack() as ctx:
                    def src_t(x_t, i, rows):
                        nc.sync.dma_start(x_t[:rows],
                                          x_in[1 + i * P:1 + i * P + rows, :])
                    vchunks = [
                        ((lambda st: (lambda chlen, half:
                            v_t[0:chlen, st, half * 8:half * 8 + (8, 4)[half],
                                0:D]))(st),
                         st * P, min(P, NT - st * P))
                        for st in range(13)]
                    qkv_phase(nc, tc, ctx, src_t, NT, wqk_t, wv_t,
                              qk_t, v_t, vchunks, ident, eps_t)
            sWA.close()

            sWB = ExitStack()   # wproj_t, wtfc       [ph2 .. ph3]
            wB = sWB.enter_context(tc.tile_pool(name="wB", bufs=1))
            wptfc = wB.tile([P, CO, C], BF)
            nc.gpsimd.dma_start(wptfc,
                                w_ptfc[:].rearrange("(co p) o -> p co o", p=P))
            sOT = ExitStack()   # oT_t                [ph2 .. ph3]
            otp_ = sOT.enter_context(tc.tile_pool(name="oT_t_pool", bufs=1))
            oT_t = otp_.tile([P, CO, NT], BF)

            # prefetch phase-4 weights during phases 2-3
            wqk_s = wC.tile([P, CO, 2 * C], BF)
            nc.gpsimd.dma_start(wqk_s, w_qk_s[:].rearrange("(co p) o -> p co o", p=P))
            wv_s = wC.tile([P, CO, C], BF)
            nc.gpsimd.dma_start(wv_s, w_v_s[:].rearrange("(co p) o -> p co o", p=P))

            if "2" in PHASES:
                phase2_temporal_attn(nc, tc, qk_t, v_t, m01, sel_sb, oT_t)
                if KDEBUG:
                    nc.sync.dma_start(dbg["d_qk_t"][:], qk_t)
                    nc.sync.dma_start(dbg["d_v_t"][:], v_t)
                    nc.sync.dma_start(dbg["d_oT_t"][:], oT_t)
            if "3" in PHASES:
                phase3_temporal_proj(nc, tc, x_in, wptfc, oT_t, xs_d)
            sOT.close()
            sWB.close()
            sPT.close()

            # ---- spatial phases 4-6 ----
            wproj_s = wD.tile([P, CO, C], BF)
            nc.gpsimd.dma_start(wproj_s,
                                w_proj_s[:].rearrange("(co p) o -> p co o", p=P))
            wfc1 = wL.tile([P, CO, HID], BF)
            nc.gpsimd.dma_start(wfc1, w_fc1[:].rearrange("(co p) o -> p co o", p=P))

            sPS = ExitStack()   # qk_s, v_s, oT_s     [ph4 .. ph6]
            poolS = sPS.enter_context(tc.tile_pool(name="poolS", bufs=1))
            qk_s = poolS.tile([P, 12, NS], BF)
            v_s = poolS.tile([P, 8, 2, 12, D + 1], BF)
            nc.gpsimd.memset(v_s[:, :, :, :, D:D + 1], 1.0)
            oT_s = poolS.tile([P, CO, NS], BF)

            if "4" in PHASES:
                with ExitStack() as ctx:
                    xs_flat = xs_d[:].rearrange("f n c -> (f n) c")

                    def src_s(x_t, i, rows):
                        nc.sync.dma_start(x_t[:rows],
                                          xs_flat[i * P:i * P + rows, :])
                    vchunks = [
                        ((lambda f, chi: (lambda chlen, half:
                            v_s[0:chlen, f, chi, half * 8:half * 8 + (8, 4)[half],
                                0:D]))(f, chi),
                         f * NSEQ + chi * P, (P, NSEQ - P)[chi])
                        for f in range(8) for chi in range(2)]
                    qkv_phase(nc, tc, ctx, src_s, NS, wqk_s, wv_s,
                              qk_s, v_s, vchunks, ident, eps_t)
            if "5" in PHASES:
                phase5_spatial_attn(nc, tc, qk_s, v_s, sel_sb, oT_s)
                if KDEBUG:
                    nc.sync.dma_start(dbg["d_qk_s"][:], qk_s)
                    nc.sync.dma_start(dbg["d_v_s"][:], v_s)
                    nc.sync.dma_start(dbg["d_oT_s"][:], oT_s)
            if "6" in PHASES:
                phase6_spatial_proj(nc, tc, xs_d, wproj_s, oT_s, y_s)
            sPS.close()
            sWD.close()
            sWC.close()

            # ---- cls + MLP ----
            sWF = ExitStack()   # wfc2                [ph7 .. end]
            wF = sWF.enter_context(tc.tile_pool(name="wF", bufs=1))
            wfc2 = wF.tile([P, HIDO, C], BF)
            nc.gpsimd.dma_start(wfc2, w_fc2[:].rearrange("(ho p) o -> p ho o", p=P))

            if "7" in PHASES:
                phase7_cls(nc, tc, y_s, one8, cls_row)
            if "8" in PHASES:
                phase8_mlp(nc, tc, y_s, cls_row, wfc1, wfc2, out, ident, eps_t)
            sWF.close()
            sWL.close()

            if KDEBUG:
                nc.sync.dma_start(dbg["d_xs"][:], xs_d[:])
                nc.sync.dma_start(dbg["d_y_s"][:], y_s[:])
                nc.sync.dma_start(dbg["d_cls"][:], cls_row[:])

    nc.compile()
    return nc


_NC_CACHE = None


def _get_nc():
    global _NC_CACHE
    if _NC_CACHE is None:
        _NC_CACHE = build_nc()
    return _NC_CACHE


def make_consts():
    bf = ml_dtypes.bfloat16
    sel = np.zeros((12, C), np.float32)
    for pr in range(6):
        for p in range(P):
            sel[2 * pr + p // D, pr * P + p] = 1.0
    one8 = np.full((8, 1), 0.125, np.float32)
    m01 = np.zeros((P, P), np.float32)
    for s in range(16):
        m01[s * 8:(s + 1) * 8, s * 8:(s + 1) * 8] = 1.0
    ident = np.eye(P, dtype=np.float32)
    return sel.astype(bf), one8.astype(bf), m01.astype(bf), ident.astype(bf)


def kernel(**inputs):
    bf = ml_dtypes.bfloat16
    x = np.asarray(inputs["x"], dtype=np.float32)
    B = x.shape[0]
    tr = lambda w: np.ascontiguousarray(
        np.asarray(w, np.float32).T.astype(bf))
    qkv_w = np.asarray(inputs["qkv_w"], np.float32)
    tqkv_w = np.asarray(inputs["tqkv_w"], np.float32)
    sel, one8, m01, ident = make_consts()
    shared = {
        "w_qk_t": tr(tqkv_w[:2 * C]), "w_v_t": tr(tqkv_w[2 * C:]),
        "w_qk_s": tr(qkv_w[:2 * C]), "w_v_s": tr(qkv_w[2 * C:]),
        "w_ptfc": tr(np.asarray(inputs["tfc_w"], np.float32)
                     @ np.asarray(inputs["tproj_w"], np.float32)),
        "w_proj_s": tr(inputs["proj_w"]),
        "w_fc1": tr(inputs["fc1_w"]), "w_fc2": tr(inputs["fc2_w"]),
        "sel12": sel, "one8": one8, "m01": m01, "ident": ident,
    }
    nc = _get_nc()
    in_maps = [dict(shared, x=np.ascontiguousarray(x[b].astype(bf)))
               for b in range(B)]
    res = run_bass_kernel_spmd(nc, in_maps, core_ids=list(range(B)),
                               trace=bool(int(os.environ.get("KTRACE", "0"))))
    out = np.stack([res.results[b]["out"] for b in range(B)], axis=0)
    kernel.last_results = res
    return out


# revision 22
# speedup vs baseline: 1.7440x; 1.0484x over previous
"""TimeSformer-style block (temporal attn -> spatial attn -> MLP) on 8 trn2 cores.

Data-parallel over B=8: each NeuronCore processes one batch element end to end.
All GEMMs run in bf16 (1 cycle/row on the PE at any free-dim size, 4x cheaper
weight loads than fp32r); PSUM accumulation stays fp32. Attention q/k/v images
live in SBUF (no HBM round trips); the MLP is fused per 512-token block so the
fc1 activation image never touches DRAM. Block-diagonal attention masking uses
a multiplicative 0/1 mask after exp instead of extra contraction rows.
"""

import os
import sys
from contextlib import ExitStack

sys.path.insert(0, "/opt/trn_rl_repo")

import numpy as np
import ml_dtypes

import concourse.bass as bass
import concourse.mybir as mybir
import concourse.tile as tile
from concourse import bacc
from concourse.bass_utils import run_bass_kernel_spmd

F32 = mybir.dt.float32
BF = mybir.dt.bfloat16
AF = mybir.ActivationFunctionType
ALU = mybir.AluOpType

C = 768
CO = 6           # C / 128
H = 12
D = 64
T = 8
G = 196          # h*w sequences
NT = G * T       # 1568 temporal tokens
NSEQ = 197       # spatial seq len (cls + 196)
NS = 8 * NSEQ    # 1576 spatial tokens
N = 1569
HID = 3072
HIDO = 24        # HID / 128
P = 128
EPS = 1e-5
SCALE = D ** -0.5

PHASES = os.environ.get("KPHASES", "12345678")
KDEBUG = bool(int(os.environ.get("KDEBUG", "0")))


def _ceil(a, b):
    return (a + b - 1) // b


def ln_tile(nc, tmp, out_pool, x_t, rows, eps_t):
    """LayerNorm over free dim (768) of a [rows<=128, 768] token-major tile."""
    stats = tmp.tile([P, 3, 6], F32, tag="ln_stats")
    for s in range(3):
        nc.vector.bn_stats(out=stats[:rows, s], in_=x_t[:rows, s * 256:(s + 1) * 256])
    mv = tmp.tile([P, 2], F32, tag="ln_mv")
    nc.vector.bn_aggr(out=mv[:rows], in_=stats[:rows])
    nc.scalar.activation(out=mv[:rows, 1:2], in_=mv[:rows, 1:2], func=AF.Sqrt,
                         bias=eps_t[:rows], scale=1.0)
    nc.vector.reciprocal(out=mv[:rows, 1:2], in_=mv[:rows, 1:2])
    xln = out_pool.tile([P, C], BF, tag="ln_out")
    nc.vector.tensor_scalar(out=xln[:rows], in0=x_t[:rows],
                            scalar1=mv[:rows, 0:1], scalar2=mv[:rows, 1:2],
                            op0=ALU.subtract, op1=ALU.mult)
    return xln


def qkv_phase(nc, tc, ctx, src_dma, ntok, wqk, wv, qk_img, v_img, vchunks,
              ident, eps_t):
    """LN -> PE transpose -> qk (c-major SBUF image) + v (token-major chunks)."""
    lnp = ctx.enter_context(tc.tile_pool(name="lnp", bufs=3))
    lnout = ctx.enter_context(tc.tile_pool(name="lnout", bufs=3))
    tpp = ctx.enter_context(tc.tile_pool(name="tp_ps", bufs=3, space="PSUM"))
    xlp = ctx.enter_context(tc.tile_pool(name="xlnT", bufs=1))
    mmp = ctx.enter_context(tc.tile_pool(name="mm_ps", bufs=4, space="PSUM"))

    xlnT = xlp.tile([P, CO, ntok], BF)
    ntiles = _ceil(ntok, P)
    for i in range(ntiles):
        rows = min(P, ntok - i * P)
        x_t = lnp.tile([P, C], BF, tag="x_t")
        src_dma(x_t, i, rows)
        xln = ln_tile(nc, lnp, lnout, x_t, rows, eps_t)
        for cp in range(CO // 2):
            pt = tpp.tile([P, 2, P], BF, tag="tp")
            for k in range(2):
                co = 2 * cp + k
                nc.tensor.transpose(pt[:, k, :rows],
                                    xln[:rows, co * P:(co + 1) * P],
                                    ident[:rows, :rows])
            nc.vector.tensor_copy(
                out=xlnT[:, 2 * cp:2 * cp + 2, i * P:i * P + rows],
                in_=pt[:, :, :rows])
    for b in range(_ceil(ntok, 512)):
        cols = min(512, ntok - b * 512)
        for o in range(12):  # 2C/128 output chunks (q then k)
            ps = mmp.tile([P, 512], F32, tag="mm")
            for co in range(CO):
                nc.tensor.matmul(ps[:, :cols], wqk[:, co, o * P:(o + 1) * P],
                                 xlnT[:, co, b * 512:b * 512 + cols],
                                 start=(co == 0), stop=(co == CO - 1))
            nc.scalar.copy(out=qk_img[:, o, b * 512:b * 512 + cols],
                           in_=ps[:, :cols])
    for dst_fn, col0v, chlen in vchunks:
        for half, fcols in ((0, 512), (1, 256)):
            ps = mmp.tile([P, 512], F32, tag="mm")
            for co in range(CO):
                nc.tensor.matmul(ps[:chlen, :fcols],
                                 xlnT[:, co, col0v:col0v + chlen],
                                 wv[:, co, half * 512:half * 512 + fcols],
                                 start=(co == 0), stop=(co == CO - 1))
            nc.vector.tensor_copy(out=dst_fn(chlen, half), in_=ps[:chlen, :fcols])


def phase2_temporal_attn(nc, tc, qk_t, v_t, m01_2, sel_sb, oT_t):
    """Subtile-outer with lag-1 AV issue and per-subtile normalize; every
    matmul writes its PSUM tile at offset 0 (free-offset accumulation hangs
    the device)."""
    with ExitStack() as ctx:
        sp = ctx.enter_context(tc.tile_pool(name="t_sps", bufs=4, space="PSUM"))
        op = ctx.enter_context(tc.tile_pool(name="t_ops", bufs=2, space="PSUM"))
        pp = ctx.enter_context(tc.tile_pool(name="t_p", bufs=8))
        sig = ctx.enter_context(tc.tile_pool(name="t_sig", bufs=1))
        bcp = ctx.enter_context(tc.tile_pool(name="t_bc", bufs=2, space="PSUM"))

        sigma = sig.tile([12, NT], F32)
        rinv = sig.tile([12, NT], BF)
        nst = _ceil(NT, P)

        def attn_head(st, h, rows, pt):
            po = op.tile([D + 1, P], F32, tag="o_ps")
            nc.tensor.matmul(po[:, :rows], v_t[0:rows, st, h, 0:D + 1],
                             pt[:rows, :rows], start=True, stop=True)
            hp, hc = (h % 2) * D, h // 2
            nc.vector.tensor_copy(out=oT_t[hp:hp + D, hc, st * P:st * P + rows],
                                  in_=po[0:D, :rows])
            sgst = pp.tile([1, P], F32, tag="sg_st")
            nc.scalar.copy(out=sgst, in_=po[D:D + 1])
            nc.sync.dma_start(sigma[h:h + 1, st * P:st * P + rows],
                              sgst[0:1, :rows])

        def normalize(st, rows):
            with nc.allow_low_precision(reason="rinv bf16 feeds bcast matmul"):
                nc.vector.reciprocal(out=rinv[:, st * P:st * P + rows],
                                     in_=sigma[:, st * P:st * P + rows])
            for pr in range(CO):
                bc = bcp.tile([P, P], F32, tag="bc_ps")
                nc.tensor.matmul(bc[:, :rows], sel_sb[:, pr * P:(pr + 1) * P],
                                 rinv[:, st * P:st * P + rows],
                                 start=True, stop=True)
                nc.vector.tensor_mul(out=oT_t[:, pr, st * P:st * P + rows],
                                     in0=oT_t[:, pr, st * P:st * P + rows],
                                     in1=bc[:, :rows])

        pend = None
        for st in range(nst):
            rows = min(P, NT - st * P)
            for h in range(H):
                hp, hc = (h % 2) * D, h // 2
                ps = sp.tile([P, P], F32, tag="s_ps")
                nc.tensor.matmul(ps[:rows, :rows],
                                 qk_t[hp:hp + D, 6 + hc, st * P:st * P + rows],
                                 qk_t[hp:hp + D, hc, st * P:st * P + rows],
                                 start=True, stop=True)
                pe = pp.tile([P, P], BF, tag="p_e")
                nc.scalar.activation(out=pe[:rows], in_=ps[:rows],
                                     func=AF.Exp, scale=SCALE)
                pt = pp.tile([P, P], BF, tag="p_t")
                eng = nc.gpsimd if h % 2 else nc.vector
                eng.tensor_mul(out=pt[:rows], in0=pe[:rows],
                               in1=m01_2[:rows, 0])
                if pend is not None:
                    attn_head(*pend)
                pend = (st, h, rows, pt)
            if st > 0:
                normalize(st - 1, P)
        attn_head(*pend)
        normalize(nst - 1, min(P, NT - (nst - 1) * P))


def phase3_temporal_proj(nc, tc, x_in, wptfc, oT_t, xs_d):
    """xt = x + oT @ (tfc_w @ tproj_w).T, scattered to spatial layout."""
    with ExitStack() as ctx:
        mp = ctx.enter_context(tc.tile_pool(name="p3_ps", bufs=4, space="PSUM"))
        tp = ctx.enter_context(tc.tile_pool(name="p3_t", bufs=4))

        # xs_d[f, 0, :] = x[0] (cls) for every frame
        cls_sb = tp.tile([8, C], BF, tag="cls_sb")
        nc.gpsimd.dma_start(cls_sb, bass.AP(tensor=x_in, offset=0,
                                            ap=[[0, 8], [1, C]]))
        nc.sync.dma_start(xs_d[:, 0, :], cls_sb)

        for i in range(_ceil(NT, P)):
            tok0 = i * P
            rows = min(P, NT - tok0)
            x_t = tp.tile([P, C], BF, tag="x_t3")
            nc.sync.dma_start(x_t[:rows], x_in[1 + tok0:1 + tok0 + rows, :])
            xt = tp.tile([P, C], BF, tag="xt3")
            for half, fcols in ((0, 512), (1, 256)):
                ps = mp.tile([P, 512], F32, tag="p3ps")
                for co in range(CO):
                    nc.tensor.matmul(
                        ps[:rows, :fcols], oT_t[:, co, tok0:tok0 + rows],
                        wptfc[:, co, half * 512:half * 512 + fcols],
                        start=(co == 0), stop=(co == CO - 1))
                nc.vector.tensor_add(
                    out=xt[:rows, half * 512:half * 512 + fcols],
                    in0=ps[:rows, :fcols],
                    in1=x_t[:rows, half * 512:half * 512 + fcols])
            # token g*8+t -> xs_d[t, 1+g]; permutation on the DRAM-side AP
            g0 = tok0 // T
            ng = rows // T
            nc.sync.dma_start(
                xs_d[:, 1 + g0:1 + g0 + ng, :].rearrange("t g c -> g t c"),
                xt[:rows])


def phase5_spatial_attn(nc, tc, qk_s, v_s, sel_sb, oT_s):
    """Per-frame attention with lag-1 AV issue and frame-lagged normalize.
    All PSUM matmul outputs at tile offset 0."""
    with ExitStack() as ctx:
        sp = ctx.enter_context(tc.tile_pool(name="s_sps", bufs=4, space="PSUM"))
        op = ctx.enter_context(tc.tile_pool(name="s_ops", bufs=2, space="PSUM"))
        pp = ctx.enter_context(tc.tile_pool(name="s_p", bufs=6))
        sig = ctx.enter_context(tc.tile_pool(name="s_sig", bufs=2))
        bcp = ctx.enter_context(tc.tile_pool(name="s_bc", bufs=2, space="PSUM"))

        CHUNKS = ((0, 0, P), (1, P, NSEQ - P))

        def do_av(f, h, pts, sigma):
            hp, hc = (h % 2) * D, h // 2
            col0 = f * NSEQ
            po = op.tile([D + 1, NSEQ], F32, tag="o_ps_s")
            for ch, off, chlen in CHUNKS:
                nc.tensor.matmul(po, v_s[0:chlen, f, ch, h, 0:D + 1],
                                 pts[ch][:chlen], start=(ch == 0),
                                 stop=(ch == 1))
            nc.vector.tensor_copy(out=oT_s[hp:hp + D, hc, col0:col0 + NSEQ],
                                  in_=po[0:D])
            sgst = pp.tile([1, NSEQ], F32, tag="sg_st")
            nc.scalar.copy(out=sgst, in_=po[D:D + 1])
            nc.sync.dma_start(sigma[h:h + 1], sgst)

        def normalize(f, sigma, rinv):
            col0 = f * NSEQ
            with nc.allow_low_precision(reason="rinv bf16 feeds bcast matmul"):
                nc.vector.reciprocal(out=rinv, in_=sigma)
            for pr in range(CO):
                bc = bcp.tile([P, NSEQ], F32, tag="bc_s")
                nc.tensor.matmul(bc, sel_sb[:, pr * P:(pr + 1) * P], rinv,
                                 start=True, stop=True)
                nc.vector.tensor_mul(out=oT_s[:, pr, col0:col0 + NSEQ],
                                     in0=oT_s[:, pr, col0:col0 + NSEQ], in1=bc)

        pend = None
        prev_sig = None
        for f in range(8):
            col0 = f * NSEQ
            sigma = sig.tile([12, NSEQ], F32, tag="sig_s")
            rinv = sig.tile([12, NSEQ], BF, tag="rinv_s")
            for h in range(H):
                hp, hc = (h % 2) * D, h // 2
                pts = []
                for ch, off, chlen in CHUNKS:
                    ps = sp.tile([P, NSEQ], F32, tag="s_ps_s")
                    nc.tensor.matmul(
                        ps[:chlen],
                        qk_s[hp:hp + D, 6 + hc, col0 + off:col0 + off + chlen],
                        qk_s[hp:hp + D, hc, col0:col0 + NSEQ],
                        start=True, stop=True)
                    pt = pp.tile([P, NSEQ], BF, tag="p_s")
                    nc.scalar.activation(out=pt[:chlen], in_=ps[:chlen],
                                         func=AF.Exp, scale=SCALE)
                    pts.append(pt)
                if pend is not None:
                    do_av(*pend)
                pend = (f, h, pts, sigma)
            if prev_sig is not None:
                normalize(f - 1, *prev_sig)
            prev_sig = (sigma, rinv)
        do_av(*pend)
        normalize(7, *prev_sig)


def phase6_spatial_proj(nc, tc, xs_d, wproj_s, oT_s, y_s):
    with ExitStack() as ctx:
        mp = ctx.enter_context(tc.tile_pool(name="p6_ps", bufs=4, space="PSUM"))
        tp = ctx.enter_context(tc.tile_pool(name="p6_t", bufs=4))
        xs_flat = xs_d[:].rearrange("f n c -> (f n) c")
        ys_flat = y_s[:].rearrange("f n c -> (f n) c")
        for i in range(_ceil(NS, P)):
            rows = min(P, NS - i * P)
            x_t = tp.tile([P, C], BF, tag="x_t6")
            nc.sync.dma_start(x_t[:rows], xs_flat[i * P:i * P + rows, :])
            yt = tp.tile([P, C], BF, tag="yt6")
            for half, fcols in ((0, 512), (1, 256)):
                ps = mp.tile([P, 512], F32, tag="p6ps")
                for co in range(CO):
                    nc.tensor.matmul(
                        ps[:rows, :fcols], oT_s[:, co, i * P:i * P + rows],
                        wproj_s[:, co, half * 512:half * 512 + fcols],
                        start=(co == 0), stop=(co == CO - 1))
                nc.vector.tensor_add(
                    out=yt[:rows, half * 512:half * 512 + fcols],
                    in0=ps[:rows, :fcols],
                    in1=x_t[:rows, half * 512:half * 512 + fcols])
            nc.sync.dma_start(ys_flat[i * P:i * P + rows, :], yt[:rows])


def phase7_cls(nc, tc, y_s, one8_in, cls_row):
    with ExitStack() as ctx:
        tp = ctx.enter_context(tc.tile_pool(name="p7_t", bufs=1))
        mp = ctx.enter_context(tc.tile_pool(name="p7_ps", bufs=2, space="PSUM"))
        ycls = tp.tile([8, C], BF)
        nc.sync.dma_start(ycls, y_s[:, 0, :])
        o8 = tp.tile([8, 1], BF)
        nc.sync.dma_start(o8, one8_in[:])
        res = tp.tile([1, C], BF)
        for half, fcols in ((0, 512), (1, 256)):
            ps = mp.tile([1, 512], F32, tag="p7_ps")
            nc.tensor.matmul(ps[:, :fcols], o8,
                             ycls[:, half * 512:half * 512 + fcols],
                             start=True, stop=True)
            nc.vector.tensor_copy(out=res[:, half * 512:half * 512 + fcols],
                                  in_=ps[:, :fcols])
        nc.sync.dma_start(cls_row[:], res)


def phase8_mlp(nc, tc, y_s, cls_row, wfc1, wfc2, out, ident, eps_t):
    blocks = [(0, 512), (512, 512), (1024, 512), (1536, 33)]
    with ExitStack() as ctx:
        lnp = ctx.enter_context(tc.tile_pool(name="p8_ln", bufs=3))
        lnout = ctx.enter_context(tc.tile_pool(name="p8_lno", bufs=3))
        xrp = ctx.enter_context(tc.tile_pool(name="p8_xr", bufs=5))
        tpp = ctx.enter_context(tc.tile_pool(name="p8_tp", bufs=3, space="PSUM"))
        xlp = ctx.enter_context(tc.tile_pool(name="p8_xlT", bufs=2))
        mp = ctx.enter_context(tc.tile_pool(name="p8_ps", bufs=4, space="PSUM"))
        h1p = ctx.enter_context(tc.tile_pool(name="p8_h1", bufs=2))
        otp = ctx.enter_context(tc.tile_pool(name="p8_o", bufs=3))

        for tok0, btok in blocks:
            nits = _ceil(btok, P)
            xlnT = xlp.tile([P, CO, 512], BF, tag="xlnT8")
            xts = []
            for it in range(nits):
                rows = min(P, btok - it * P)
                x_t = xrp.tile([P, C], BF, tag="x_t8")
                g0 = (tok0 + it * P) // T
                if btok == 33:
                    nc.sync.dma_start(
                        x_t[:32],
                        y_s[:, 1 + g0:1 + g0 + 4, :].rearrange("t g c -> g t c"))
                    nc.sync.dma_start(x_t[32:33], cls_row[:])
                else:
                    nc.sync.dma_start(
                        x_t[:rows],
                        y_s[:, 1 + g0:1 + g0 + 16, :].rearrange("t g c -> g t c"))
                xts.append((x_t, rows))
                xln = ln_tile(nc, lnp, lnout, x_t, rows, eps_t)
                for cp in range(CO // 2):
                    pt = tpp.tile([P, 2, P], BF, tag="tp8")
                    for k in range(2):
                        co = 2 * cp + k
                        nc.tensor.transpose(pt[:, k, :rows],
                                            xln[:rows, co * P:(co + 1) * P],
                                            ident[:rows, :rows])
                    nc.vector.tensor_copy(
                        out=xlnT[:, 2 * cp:2 * cp + 2, it * P:it * P + rows],
                        in_=pt[:, :, :rows])
            h1 = h1p.tile([P, HIDO, 512], BF, tag="h1blk")
            for o in range(HIDO):
                ps = mp.tile([P, 512], F32, tag="p8ps")
                for co in range(CO):
                    nc.tensor.matmul(ps[:, :btok], wfc1[:, co, o * P:(o + 1) * P],
                                     xlnT[:, co, :btok],
                                     start=(co == 0), stop=(co == CO - 1))
                nc.scalar.activation(out=h1[:, o, :btok], in_=ps[:, :btok],
                                     func=AF.Gelu)
            for it in range(nits):
                x_t, rows = xts[it]
                o_t = otp.tile([P, C], F32, tag="o_t8")
                for half, fcols in ((0, 512), (1, 256)):
                    ps = mp.tile([P, 512], F32, tag="p8ps")
                    for ho in range(HIDO):
                        nc.tensor.matmul(
                            ps[:rows, :fcols], h1[:, ho, it * P:it * P + rows],
                            wfc2[:, ho, half * 512:half * 512 + fcols],
                            start=(ho == 0), stop=(ho == HIDO - 1))
                    nc.vector.tensor_add(
                        out=o_t[:rows, half * 512:half * 512 + fcols],
                        in0=ps[:rows, :fcols],
                        in1=x_t[:rows, half * 512:half * 512 + fcols])
                row0 = tok0 + it * P
                if btok == 33:
                    nc.sync.dma_start(out[1 + row0:1 + row0 + 32, :], o_t[:32])
                    nc.sync.dma_start(out[0:1, :], o_t[32:33])
                else:
                    nc.sync.dma_start(out[1 + row0:1 + row0 + rows, :],
                                      o_t[:rows])


def build_nc():
    nc = bacc.Bacc("TRN2", target_bir_lowering=False, debug=False)

    x_in = nc.dram_tensor("x", (N, C), BF, kind="ExternalInput")
    w_qk_t = nc.dram_tensor("w_qk_t", (C, 2 * C), BF, kind="ExternalInput")
    w_v_t = nc.dram_tensor("w_v_t", (C, C), BF, kind="ExternalInput")
    w_qk_s = nc.dram_tensor("w_qk_s", (C, 2 * C), BF, kind="ExternalInput")
    w_v_s = nc.dram_tensor("w_v_s", (C, C), BF, kind="ExternalInput")
    w_ptfc = nc.dram_tensor("w_ptfc", (C, C), BF, kind="ExternalInput")
    w_proj_s = nc.dram_tensor("w_proj_s", (C, C), BF, kind="ExternalInput")
    w_fc1 = nc.dram_tensor("w_fc1", (C, HID), BF, kind="ExternalInput")
    w_fc2 = nc.dram_tensor("w_fc2", (HID, C), BF, kind="ExternalInput")
    sel12 = nc.dram_tensor("sel12", (12, C), BF, kind="ExternalInput")
    one8 = nc.dram_tensor("one8", (8, 1), BF, kind="ExternalInput")
    m01_in = nc.dram_tensor("m01", (P, 2, P), BF, kind="ExternalInput")
    ident_in = nc.dram_tensor("ident", (P, P), BF, kind="ExternalInput")
    out = nc.dram_tensor("out", (N, C), F32, kind="ExternalOutput")
    dbg = {}
    if KDEBUG:
        for nm, shp in (("d_qk_t", (P, 12, NT)), ("d_v_t", (P, 13, 12, D + 1)),
                        ("d_oT_t", (P, CO, NT)), ("d_xs", (8, NSEQ, C)),
                        ("d_qk_s", (P, 12, NS)),
                        ("d_v_s", (P, 8, 2, 12, D + 1)),
                        ("d_oT_s", (P, CO, NS)), ("d_y_s", (8, NSEQ, C)),
                        ("d_cls", (1, C))):
            dbg[nm] = nc.dram_tensor(nm, shp, BF, kind="ExternalOutput")

    with tile.TileContext(nc) as tc:
        with ExitStack() as root:
            dram = root.enter_context(tc.tile_pool(name="dram", bufs=1,
                                                   space="DRAM"))
            const = root.enter_context(tc.tile_pool(name="const", bufs=1))
            xs_d = dram.tile([8, NSEQ, C], BF)
            y_s = dram.tile([8, NSEQ, C], BF)
            cls_row = dram.tile([1, C], BF)

            ident = const.tile([P, P], BF)
            nc.sync.dma_start(ident, ident_in[:])
            m01 = const.tile([P, 2, P], BF)
            nc.sync.dma_start(m01, m01_in[:])
            eps_t = const.tile([P, 1], F32)
            nc.vector.memset(eps_t, EPS)
            sel_sb = const.tile([12, C], BF)
            nc.sync.dma_start(sel_sb, sel12[:])

            # Stack allocator: a pool reserves its full size at open, so
            # lifetimes must nest (LIFO).  Onion order, outermost first;
            # tiles/DMAs are issued later at the prefetch point.
            sWL = ExitStack()   # wfc1                [t0 .. end]
            wL = sWL.enter_context(tc.tile_pool(name="wL", bufs=1))
            sWC = ExitStack()   # wqk_s, wv_s         [t0 .. ph6]
            wC = sWC.enter_context(tc.tile_pool(name="wC", bufs=1))
            sWD = ExitStack()   # wproj_s             [t0 .. ph6]
            wD = sWD.enter_context(tc.tile_pool(name="wD", bufs=1))

            # ---- temporal phases 1-3 ----
            sPT = ExitStack()   # qk_t, v_t           [ph1 .. ph3]
            poolT = sPT.enter_context(tc.tile_pool(name="poolT", bufs=1))
            qk_t = poolT.tile([P, 12, NT], BF)
            v_t = poolT.tile([P, 13, 12, D + 1], BF)
            nc.gpsimd.memset(v_t[:, :, :, D:D + 1], 1.0)

            sWA = ExitStack()   # wqk_t, wv_t         [ph1]
            wA = sWA.enter_context(tc.tile_pool(name="wA", bufs=1))
            wqk_t = wA.tile([P, CO, 2 * C], BF)
            nc.sync.dma_start(wqk_t, w_qk_t[:].rearrange("(co p) o -> p co o", p=P))
            wv_t = wA.tile([P, CO, C], BF)
            nc.sync.dma_start(wv_t, w_v_t[:].rearrange("(co p) o -> p co o", p=P))

            if "1" in PHASES:
                with ExitStack() as ctx:
                    def src_t(x_t, i, rows):
                        nc.sync.dma_start(x_t[:rows],
                                          x_in[1 + i * P:1 + i * P + rows, :])
                    vchunks = [
                        ((lambda st: (lambda chlen, half:
                            v_t[0:chlen, st, half * 8:half * 8 + (8, 4)[half],
                                0:D]))(st),
                         st * P, min(P, NT - st * P))
                        for st in range(13)]
                    qkv_phase(nc, tc, ctx, src_t, NT, wqk_t, wv_t,
                              qk_t, v_t, vchunks, ident, eps_t)
            sWA.close()

            sWB = ExitStack()   # wproj_t, wtfc       [ph2 .. ph3]
            wB = sWB.enter_context(tc.tile_pool(name="wB", bufs=1))
            wptfc = wB.tile([P, CO, C], BF)
            nc.sync.dma_start(wptfc,
                                w_ptfc[:].rearrange("(co p) o -> p co o", p=P))
            sOT = ExitStack()   # oT_t                [ph2 .. ph3]
            otp_ = sOT.enter_context(tc.tile_pool(name="oT_t_pool", bufs=1))
            oT_t = otp_.tile([P, CO, NT], BF)

            # prefetch phase-4 weights during phases 2-3
            wqk_s = wC.tile([P, CO, 2 * C], BF)
            nc.sync.dma_start(wqk_s, w_qk_s[:].rearrange("(co p) o -> p co o", p=P))
            wv_s = wC.tile([P, CO, C], BF)
            nc.sync.dma_start(wv_s, w_v_s[:].rearrange("(co p) o -> p co o", p=P))

            if "2" in PHASES:
                phase2_temporal_attn(nc, tc, qk_t, v_t, m01, sel_sb, oT_t)
                if KDEBUG:
                    nc.sync.dma_start(dbg["d_qk_t"][:], qk_t)
                    nc.sync.dma_start(dbg["d_v_t"][:], v_t)
                    nc.sync.dma_start(dbg["d_oT_t"][:], oT_t)
            if "3" in PHASES:
                phase3_temporal_proj(nc, tc, x_in, wptfc, oT_t, xs_d)
            sOT.close()
            sWB.close()
            sPT.close()

            # ---- spatial phases 4-6 ----
            wproj_s = wD.tile([P, CO, C], BF)
            nc.sync.dma_start(wproj_s,
                                w_proj_s[:].rearrange("(co p) o -> p co o", p=P))
            wfc1 = wL.tile([P, CO, HID], BF)
            nc.sync.dma_start(wfc1, w_fc1[:].rearrange("(co p) o -> p co o", p=P))

            sPS = ExitStack()   # qk_s, v_s, oT_s     [ph4 .. ph6]
            poolS = sPS.enter_context(tc.tile_pool(name="poolS", bufs=1))
            qk_s = poolS.tile([P, 12, NS], BF)
            v_s = poolS.tile([P, 8, 2, 12, D + 1], BF)
            nc.gpsimd.memset(v_s[:, :, :, :, D:D + 1], 1.0)
            oT_s = poolS.tile([P, CO, NS], BF)

            if "4" in PHASES:
                with ExitStack() as ctx:
                    xs_flat = xs_d[:].rearrange("f n c -> (f n) c")

                    def src_s(x_t, i, rows):
                        nc.sync.dma_start(x_t[:rows],
                                          xs_flat[i * P:i * P + rows, :])
                    vchunks = [
                        ((lambda f, chi: (lambda chlen, half:
                            v_s[0:chlen, f, chi, half * 8:half * 8 + (8, 4)[half],
                                0:D]))(f, chi),
                         f * NSEQ + chi * P, (P, NSEQ - P)[chi])
                        for f in range(8) for chi in range(2)]
                    qkv_phase(nc, tc, ctx, src_s, NS, wqk_s, wv_s,
                              qk_s, v_s, vchunks, ident, eps_t)
            if "5" in PHASES:
                phase5_spatial_attn(nc, tc, qk_s, v_s, sel_sb, oT_s)
                if KDEBUG:
                    nc.sync.dma_start(dbg["d_qk_s"][:], qk_s)
                    nc.sync.dma_start(dbg["d_v_s"][:], v_s)
                    nc.sync.dma_start(dbg["d_oT_s"][:], oT_s)
            if "6" in PHASES:
                phase6_spatial_proj(nc, tc, xs_d, wproj_s, oT_s, y_s)
            sPS.close()
            sWD.close()
            sWC.close()

            # ---- cls + MLP ----
            sWF = ExitStack()   # wfc2                [ph7 .. end]
            wF = sWF.enter_context(tc.tile_pool(name="wF", bufs=1))
            wfc2 = wF.tile([P, HIDO, C], BF)
            nc.sync.dma_start(wfc2, w_fc2[:].rearrange("(ho p) o -> p ho o", p=P))

            if "7" in PHASES:
                phase7_cls(nc, tc, y_s, one8, cls_row)
            if "8" in PHASES:
                phase8_mlp(nc, tc, y_s, cls_row, wfc1, wfc2, out, ident, eps_t)
            sWF.close()
            sWL.close()

            if KDEBUG:
                nc.sync.dma_start(dbg["d_xs"][:], xs_d[:])
                nc.sync.dma_start(dbg["d_y_s"][:], y_s[:])
                nc.sync.dma_start(dbg["d_cls"][:], cls_row[:])

    nc.compile()
    return nc


_NC_CACHE = None


def _get_nc():
    global _NC_CACHE
    if _NC_CACHE is None:
        _NC_CACHE = build_nc()
    return _NC_CACHE


def make_consts():
    bf = ml_dtypes.bfloat16
    sel = np.zeros((12, C), np.float32)
    for pr in range(6):
        for p in range(P):
            sel[2 * pr + p // D, pr * P + p] = 1.0
    one8 = np.full((8, 1), 0.125, np.float32)
    m01 = np.zeros((P, P), np.float32)
    for s in range(16):
        m01[s * 8:(s + 1) * 8, s * 8:(s + 1) * 8] = 1.0
    m01 = np.repeat(m01[:, None, :], 2, axis=1)
    ident = np.eye(P, dtype=np.float32)
    return sel.astype(bf), one8.astype(bf), m01.astype(bf), ident.astype(bf)


def kernel(**inputs):
    bf = ml_dtypes.bfloat16
    x = np.asarray(inputs["x"], dtype=np.float32)
    B = x.shape[0]
    tr = lambda w: np.ascontiguousarray(
        np.asarray(w, np.float32).T.astype(bf))
    qkv_w = np.asarray(inputs["qkv_w"], np.float32)
    tqkv_w = np.asarray(inputs["tqkv_w"], np.float32)
    sel, one8, m01, ident = make_consts()
    shared = {
        "w_qk_t": tr(tqkv_w[:2 * C]), "w_v_t": tr(tqkv_w[2 * C:]),
        "w_qk_s": tr(qkv_w[:2 * C]), "w_v_s": tr(qkv_w[2 * C:]),
        "w_ptfc": tr(np.asarray(inputs["tfc_w"], np.float32)
                     @ np.asarray(inputs["tproj_w"], np.float32)),
        "w_proj_s": tr(inputs["proj_w"]),
        "w_fc1": tr(inputs["fc1_w"]), "w_fc2": tr(inputs["fc2_w"]),
        "sel12": sel, "one8": one8, "m01": m01, "ident": ident,
    }
    nc = _get_nc()
    in_maps = [dict(shared, x=np.ascontiguousarray(x[b].astype(bf)))
               for b in range(B)]
    res = run_bass_kernel_spmd(nc, in_maps, core_ids=list(range(B)),
                               trace=bool(int(os.environ.get("KTRACE", "0"))))
    out = np.stack([res.results[b]["out"] for b in range(B)], axis=0)
    kernel.last_results = res
    return out


# revision 23
# speedup vs baseline: 1.7606x; 1.0095x over previous
"""TimeSformer-style block (temporal attn -> spatial attn -> MLP) on 8 trn2 cores.

Data-parallel over B=8: each NeuronCore processes one batch element end to end.
All GEMMs run in bf16 (1 cycle/row on the PE at any free-dim size, 4x cheaper
weight loads than fp32r); PSUM accumulation stays fp32. Attention q/k/v images
live in SBUF (no HBM round trips); the MLP is fused per 512-token block so the
fc1 activation image never touches DRAM. Block-diagonal attention masking uses
a multiplicative 0/1 mask after exp instead of extra contraction rows.
"""

import os
import sys
from contextlib import ExitStack

sys.path.insert(0, "/opt/trn_rl_repo")

import numpy as np
import ml_dtypes

import concourse.bass as bass
import concourse.mybir as mybir
import concourse.tile as tile
from concourse import bacc
from concourse.bass_utils import run_bass_kernel_spmd

F32 = mybir.dt.float32
BF = mybir.dt.bfloat16
AF = mybir.ActivationFunctionType
ALU = mybir.AluOpType

C = 768
CO = 6           # C / 128
H = 12
D = 64
T = 8
G = 196          # h*w sequences
NT = G * T       # 1568 temporal tokens
NSEQ = 197       # spatial seq len (cls + 196)
NS = 8 * NSEQ    # 1576 spatial tokens
N = 1569
HID = 3072
HIDO = 24        # HID / 128
P = 128
EPS = 1e-5
SCALE = D ** -0.5

PHASES = os.environ.get("KPHASES", "12345678")
KDEBUG = bool(int(os.environ.get("KDEBUG", "0")))


def _ceil(a, b):
    return (a + b - 1) // b


def ln_tile(nc, tmp, out_pool, x_t, rows, eps_t):
    """LayerNorm over free dim (768) of a [rows<=128, 768] token-major tile."""
    stats = tmp.tile([P, 3, 6], F32, tag="ln_stats")
    for s in range(3):
        nc.vector.bn_stats(out=stats[:rows, s], in_=x_t[:rows, s * 256:(s + 1) * 256])
    mv = tmp.tile([P, 2], F32, tag="ln_mv")
    nc.vector.bn_aggr(out=mv[:rows], in_=stats[:rows])
    nc.scalar.activation(out=mv[:rows, 1:2], in_=mv[:rows, 1:2], func=AF.Sqrt,
                         bias=eps_t[:rows], scale=1.0)
    nc.vector.reciprocal(out=mv[:rows, 1:2], in_=mv[:rows, 1:2])
    xln = out_pool.tile([P, C], BF, tag="ln_out")
    nc.vector.tensor_scalar(out=xln[:rows], in0=x_t[:rows],
                            scalar1=mv[:rows, 0:1], scalar2=mv[:rows, 1:2],
                            op0=ALU.subtract, op1=ALU.mult)
    return xln


def qkv_phase(nc, tc, ctx, src_dma, ntok, wqk, wv, qk_img, v_img, vchunks,
              ident, eps_t, wload=None):
    """LN -> PE transpose -> qk (c-major SBUF image) + v (token-major chunks)."""
    lnp = ctx.enter_context(tc.tile_pool(name="lnp", bufs=3))
    lnout = ctx.enter_context(tc.tile_pool(name="lnout", bufs=3))
    tpp = ctx.enter_context(tc.tile_pool(name="tp_ps", bufs=3, space="PSUM"))
    xlp = ctx.enter_context(tc.tile_pool(name="xlnT", bufs=1))
    mmp = ctx.enter_context(tc.tile_pool(name="mm_ps", bufs=5, space="PSUM"))

    xlnT = xlp.tile([P, CO, ntok], BF)
    ntiles = _ceil(ntok, P)
    for i in range(ntiles):
        rows = min(P, ntok - i * P)
        x_t = lnp.tile([P, C], BF, tag="x_t")
        src_dma(x_t, i, rows)
        xln = ln_tile(nc, lnp, lnout, x_t, rows, eps_t)
        for cp in range(CO // 2):
            pt = tpp.tile([P, 2, P], BF, tag="tp")
            for k in range(2):
                co = 2 * cp + k
                nc.tensor.transpose(pt[:, k, :rows],
                                    xln[:rows, co * P:(co + 1) * P],
                                    ident[:rows, :rows])
            nc.vector.tensor_copy(
                out=xlnT[:, 2 * cp:2 * cp + 2, i * P:i * P + rows],
                in_=pt[:, :, :rows])
        if i == 0 and wload is not None:
            wload()
    for b in range(_ceil(ntok, 512)):
        cols = min(512, ntok - b * 512)
        for o in range(12):  # 2C/128 output chunks (q then k)
            ps = mmp.tile([P, 512], F32, tag="mm")
            for co in range(CO):
                nc.tensor.matmul(ps[:, :cols], wqk[:, co, o * P:(o + 1) * P],
                                 xlnT[:, co, b * 512:b * 512 + cols],
                                 start=(co == 0), stop=(co == CO - 1))
            nc.scalar.copy(out=qk_img[:, o, b * 512:b * 512 + cols],
                           in_=ps[:, :cols])
    for dst_fn, col0v, chlen in vchunks:
        for half, fcols in ((0, 512), (1, 256)):
            ps = mmp.tile([P, 512], F32, tag="mm")
            for co in range(CO):
                nc.tensor.matmul(ps[:chlen, :fcols],
                                 xlnT[:, co, col0v:col0v + chlen],
                                 wv[:, co, half * 512:half * 512 + fcols],
                                 start=(co == 0), stop=(co == CO - 1))
            nc.vector.tensor_copy(out=dst_fn(chlen, half), in_=ps[:chlen, :fcols])


def phase2_temporal_attn(nc, tc, qk_t, v_t, m01_2, sel_sb, oT_t):
    """Subtile-outer with lag-1 AV issue and per-subtile normalize; every
    matmul writes its PSUM tile at offset 0 (free-offset accumulation hangs
    the device)."""
    with ExitStack() as ctx:
        sp = ctx.enter_context(tc.tile_pool(name="t_sps", bufs=4, space="PSUM"))
        op = ctx.enter_context(tc.tile_pool(name="t_ops", bufs=2, space="PSUM"))
        pp = ctx.enter_context(tc.tile_pool(name="t_p", bufs=8))
        sig = ctx.enter_context(tc.tile_pool(name="t_sig", bufs=1))
        bcp = ctx.enter_context(tc.tile_pool(name="t_bc", bufs=2, space="PSUM"))

        sigma = sig.tile([12, NT], F32)
        rinv = sig.tile([12, NT], BF)
        nst = _ceil(NT, P)

        def attn_head(st, h, rows, pt):
            po = op.tile([D + 1, P], F32, tag="o_ps")
            nc.tensor.matmul(po[:, :rows], v_t[0:rows, st, h, 0:D + 1],
                             pt[:rows, :rows], start=True, stop=True)
            hp, hc = (h % 2) * D, h // 2
            nc.vector.tensor_copy(out=oT_t[hp:hp + D, hc, st * P:st * P + rows],
                                  in_=po[0:D, :rows])
            sgst = pp.tile([1, P], F32, tag="sg_st")
            nc.scalar.copy(out=sgst, in_=po[D:D + 1])
            nc.sync.dma_start(sigma[h:h + 1, st * P:st * P + rows],
                              sgst[0:1, :rows])

        def normalize(st, rows):
            with nc.allow_low_precision(reason="rinv bf16 feeds bcast matmul"):
                nc.vector.reciprocal(out=rinv[:, st * P:st * P + rows],
                                     in_=sigma[:, st * P:st * P + rows])
            for pr in range(CO):
                bc = bcp.tile([P, P], F32, tag="bc_ps")
                nc.tensor.matmul(bc[:, :rows], sel_sb[:, pr * P:(pr + 1) * P],
                                 rinv[:, st * P:st * P + rows],
                                 start=True, stop=True)
                nc.vector.tensor_mul(out=oT_t[:, pr, st * P:st * P + rows],
                                     in0=oT_t[:, pr, st * P:st * P + rows],
                                     in1=bc[:, :rows])

        pend = None
        for st in range(nst):
            rows = min(P, NT - st * P)
            for h in range(H):
                hp, hc = (h % 2) * D, h // 2
                ps = sp.tile([P, P], F32, tag="s_ps")
                nc.tensor.matmul(ps[:rows, :rows],
                                 qk_t[hp:hp + D, 6 + hc, st * P:st * P + rows],
                                 qk_t[hp:hp + D, hc, st * P:st * P + rows],
                                 start=True, stop=True)
                pe = pp.tile([P, P], BF, tag="p_e")
                nc.scalar.activation(out=pe[:rows], in_=ps[:rows],
                                     func=AF.Exp, scale=SCALE)
                pt = pp.tile([P, P], BF, tag="p_t")
                eng = nc.gpsimd if h % 2 else nc.vector
                eng.tensor_mul(out=pt[:rows], in0=pe[:rows],
                               in1=m01_2[:rows, 0])
                if pend is not None:
                    attn_head(*pend)
                pend = (st, h, rows, pt)
            if st > 0:
                normalize(st - 1, P)
        attn_head(*pend)
        normalize(nst - 1, min(P, NT - (nst - 1) * P))


def phase3_temporal_proj(nc, tc, x_in, wptfc, oT_t, xs_d):
    """xt = x + oT @ (tfc_w @ tproj_w).T, scattered to spatial layout."""
    with ExitStack() as ctx:
        mp = ctx.enter_context(tc.tile_pool(name="p3_ps", bufs=4, space="PSUM"))
        tp = ctx.enter_context(tc.tile_pool(name="p3_t", bufs=4))

        # xs_d[f, 0, :] = x[0] (cls) for every frame
        cls_sb = tp.tile([8, C], BF, tag="cls_sb")
        nc.gpsimd.dma_start(cls_sb, bass.AP(tensor=x_in, offset=0,
                                            ap=[[0, 8], [1, C]]))
        nc.sync.dma_start(xs_d[:, 0, :], cls_sb)

        for i in range(_ceil(NT, P)):
            tok0 = i * P
            rows = min(P, NT - tok0)
            x_t = tp.tile([P, C], BF, tag="x_t3")
            nc.sync.dma_start(x_t[:rows], x_in[1 + tok0:1 + tok0 + rows, :])
            xt = tp.tile([P, C], BF, tag="xt3")
            for half, fcols in ((0, 512), (1, 256)):
                ps = mp.tile([P, 512], F32, tag="p3ps")
                for co in range(CO):
                    nc.tensor.matmul(
                        ps[:rows, :fcols], oT_t[:, co, tok0:tok0 + rows],
                        wptfc[:, co, half * 512:half * 512 + fcols],
                        start=(co == 0), stop=(co == CO - 1))
                nc.vector.tensor_add(
                    out=xt[:rows, half * 512:half * 512 + fcols],
                    in0=ps[:rows, :fcols],
                    in1=x_t[:rows, half * 512:half * 512 + fcols])
            # token g*8+t -> xs_d[t, 1+g]; permutation on the DRAM-side AP
            g0 = tok0 // T
            ng = rows // T
            nc.sync.dma_start(
                xs_d[:, 1 + g0:1 + g0 + ng, :].rearrange("t g c -> g t c"),
                xt[:rows])


def phase5_spatial_attn(nc, tc, qk_s, v_s, sel_sb, oT_s):
    """Per-frame attention with lag-1 AV issue and frame-lagged normalize.
    All PSUM matmul outputs at tile offset 0."""
    with ExitStack() as ctx:
        sp = ctx.enter_context(tc.tile_pool(name="s_sps", bufs=4, space="PSUM"))
        op = ctx.enter_context(tc.tile_pool(name="s_ops", bufs=2, space="PSUM"))
        pp = ctx.enter_context(tc.tile_pool(name="s_p", bufs=6))
        sig = ctx.enter_context(tc.tile_pool(name="s_sig", bufs=2))
        bcp = ctx.enter_context(tc.tile_pool(name="s_bc", bufs=2, space="PSUM"))

        CHUNKS = ((0, 0, P), (1, P, NSEQ - P))

        def do_av(f, h, pts, sigma):
            hp, hc = (h % 2) * D, h // 2
            col0 = f * NSEQ
            po = op.tile([D + 1, NSEQ], F32, tag="o_ps_s")
            for ch, off, chlen in CHUNKS:
                nc.tensor.matmul(po, v_s[0:chlen, f, ch, h, 0:D + 1],
                                 pts[ch][:chlen], start=(ch == 0),
                                 stop=(ch == 1))
            nc.vector.tensor_copy(out=oT_s[hp:hp + D, hc, col0:col0 + NSEQ],
                                  in_=po[0:D])
            sgst = pp.tile([1, NSEQ], F32, tag="sg_st")
            nc.scalar.copy(out=sgst, in_=po[D:D + 1])
            nc.sync.dma_start(sigma[h:h + 1], sgst)

        def normalize(f, sigma, rinv):
            col0 = f * NSEQ
            with nc.allow_low_precision(reason="rinv bf16 feeds bcast matmul"):
                nc.vector.reciprocal(out=rinv, in_=sigma)
            for pr in range(CO):
                bc = bcp.tile([P, NSEQ], F32, tag="bc_s")
                nc.tensor.matmul(bc, sel_sb[:, pr * P:(pr + 1) * P], rinv,
                                 start=True, stop=True)
                nc.vector.tensor_mul(out=oT_s[:, pr, col0:col0 + NSEQ],
                                     in0=oT_s[:, pr, col0:col0 + NSEQ], in1=bc)

        pend = None
        prev_sig = None
        for f in range(8):
            col0 = f * NSEQ
            sigma = sig.tile([12, NSEQ], F32, tag="sig_s")
            rinv = sig.tile([12, NSEQ], BF, tag="rinv_s")
            for h in range(H):
                hp, hc = (h % 2) * D, h // 2
                pts = []
                for ch, off, chlen in CHUNKS:
                    ps = sp.tile([P, NSEQ], F32, tag="s_ps_s")
                    nc.tensor.matmul(
                        ps[:chlen],
                        qk_s[hp:hp + D, 6 + hc, col0 + off:col0 + off + chlen],
                        qk_s[hp:hp + D, hc, col0:col0 + NSEQ],
                        start=True, stop=True)
                    pt = pp.tile([P, NSEQ], BF, tag="p_s")
                    nc.scalar.activation(out=pt[:chlen], in_=ps[:chlen],
                                         func=AF.Exp, scale=SCALE)
                    pts.append(pt)
                if pend is not None:
                    do_av(*pend)
                pend = (f, h, pts, sigma)
            if prev_sig is not None:
                normalize(f - 1, *prev_sig)
            prev_sig = (sigma, rinv)
        do_av(*pend)
        normalize(7, *prev_sig)


def phase6_spatial_proj(nc, tc, xs_d, wproj_s, oT_s, y_s):
    with ExitStack() as ctx:
        mp = ctx.enter_context(tc.tile_pool(name="p6_ps", bufs=4, space="PSUM"))
        tp = ctx.enter_context(tc.tile_pool(name="p6_t", bufs=4))
        xs_flat = xs_d[:].rearrange("f n c -> (f n) c")
        ys_flat = y_s[:].rearrange("f n c -> (f n) c")
        for i in range(_ceil(NS, P)):
            rows = min(P, NS - i * P)
            x_t = tp.tile([P, C], BF, tag="x_t6")
            nc.sync.dma_start(x_t[:rows], xs_flat[i * P:i * P + rows, :])
            yt = tp.tile([P, C], BF, tag="yt6")
            for half, fcols in ((0, 512), (1, 256)):
                ps = mp.tile([P, 512], F32, tag="p6ps")
                for co in range(CO):
                    nc.tensor.matmul(
                        ps[:rows, :fcols], oT_s[:, co, i * P:i * P + rows],
                        wproj_s[:, co, half * 512:half * 512 + fcols],
                        start=(co == 0), stop=(co == CO - 1))
                nc.vector.tensor_add(
                    out=yt[:rows, half * 512:half * 512 + fcols],
                    in0=ps[:rows, :fcols],
                    in1=x_t[:rows, half * 512:half * 512 + fcols])
            nc.sync.dma_start(ys_flat[i * P:i * P + rows, :], yt[:rows])


def phase7_cls(nc, tc, y_s, one8_in, cls_row):
    with ExitStack() as ctx:
        tp = ctx.enter_context(tc.tile_pool(name="p7_t", bufs=1))
        mp = ctx.enter_context(tc.tile_pool(name="p7_ps", bufs=2, space="PSUM"))
        ycls = tp.tile([8, C], BF)
        nc.sync.dma_start(ycls, y_s[:, 0, :])
        o8 = tp.tile([8, 1], BF)
        nc.sync.dma_start(o8, one8_in[:])
        res = tp.tile([1, C], BF)
        for half, fcols in ((0, 512), (1, 256)):
            ps = mp.tile([1, 512], F32, tag="p7_ps")
            nc.tensor.matmul(ps[:, :fcols], o8,
                             ycls[:, half * 512:half * 512 + fcols],
                             start=True, stop=True)
            nc.vector.tensor_copy(out=res[:, half * 512:half * 512 + fcols],
                                  in_=ps[:, :fcols])
        nc.sync.dma_start(cls_row[:], res)


def phase8_mlp(nc, tc, y_s, cls_row, wfc1, wfc2, out, ident, eps_t,
               wload=None):
    blocks = [(0, 512), (512, 512), (1024, 512), (1536, 33)]
    with ExitStack() as ctx:
        lnp = ctx.enter_context(tc.tile_pool(name="p8_ln", bufs=3))
        lnout = ctx.enter_context(tc.tile_pool(name="p8_lno", bufs=3))
        xrp = ctx.enter_context(tc.tile_pool(name="p8_xr", bufs=5))
        tpp = ctx.enter_context(tc.tile_pool(name="p8_tp", bufs=3, space="PSUM"))
        xlp = ctx.enter_context(tc.tile_pool(name="p8_xlT", bufs=2))
        mp = ctx.enter_context(tc.tile_pool(name="p8_ps", bufs=5, space="PSUM"))
        h1p = ctx.enter_context(tc.tile_pool(name="p8_h1", bufs=2))
        otp = ctx.enter_context(tc.tile_pool(name="p8_o", bufs=3))

        for tok0, btok in blocks:
            nits = _ceil(btok, P)
            xlnT = xlp.tile([P, CO, 512], BF, tag="xlnT8")
            xts = []
            for it in range(nits):
                rows = min(P, btok - it * P)
                x_t = xrp.tile([P, C], BF, tag="x_t8")
                g0 = (tok0 + it * P) // T
                if btok == 33:
                    nc.sync.dma_start(
                        x_t[:32],
                        y_s[:, 1 + g0:1 + g0 + 4, :].rearrange("t g c -> g t c"))
                    nc.sync.dma_start(x_t[32:33], cls_row[:])
                else:
                    nc.sync.dma_start(
                        x_t[:rows],
                        y_s[:, 1 + g0:1 + g0 + 16, :].rearrange("t g c -> g t c"))
                xts.append((x_t, rows))
                if wload is not None:
                    wload()
                    wload = None
                xln = ln_tile(nc, lnp, lnout, x_t, rows, eps_t)
                for cp in range(CO // 2):
                    pt = tpp.tile([P, 2, P], BF, tag="tp8")
                    for k in range(2):
                        co = 2 * cp + k
                        nc.tensor.transpose(pt[:, k, :rows],
                                            xln[:rows, co * P:(co + 1) * P],
                                            ident[:rows, :rows])
                    nc.vector.tensor_copy(
                        out=xlnT[:, 2 * cp:2 * cp + 2, it * P:it * P + rows],
                        in_=pt[:, :, :rows])
            h1 = h1p.tile([P, HIDO, 512], BF, tag="h1blk")
            for o in range(HIDO):
                ps = mp.tile([P, 512], F32, tag="p8ps")
                for co in range(CO):
                    nc.tensor.matmul(ps[:, :btok], wfc1[:, co, o * P:(o + 1) * P],
                                     xlnT[:, co, :btok],
                                     start=(co == 0), stop=(co == CO - 1))
                nc.scalar.activation(out=h1[:, o, :btok], in_=ps[:, :btok],
                                     func=AF.Gelu)
            for it in range(nits):
                x_t, rows = xts[it]
                o_t = otp.tile([P, C], F32, tag="o_t8")
                for half, fcols in ((0, 512), (1, 256)):
                    ps = mp.tile([P, 512], F32, tag="p8ps")
                    for ho in range(HIDO):
                        nc.tensor.matmul(
                            ps[:rows, :fcols], h1[:, ho, it * P:it * P + rows],
                            wfc2[:, ho, half * 512:half * 512 + fcols],
                            start=(ho == 0), stop=(ho == HIDO - 1))
                    nc.vector.tensor_add(
                        out=o_t[:rows, half * 512:half * 512 + fcols],
                        in0=ps[:rows, :fcols],
                        in1=x_t[:rows, half * 512:half * 512 + fcols])
                row0 = tok0 + it * P
                if btok == 33:
                    nc.sync.dma_start(out[1 + row0:1 + row0 + 32, :], o_t[:32])
                    nc.sync.dma_start(out[0:1, :], o_t[32:33])
                else:
                    nc.sync.dma_start(out[1 + row0:1 + row0 + rows, :],
                                      o_t[:rows])


def build_nc():
    nc = bacc.Bacc("TRN2", target_bir_lowering=False, debug=False)

    x_in = nc.dram_tensor("x", (N, C), BF, kind="ExternalInput")
    w_qk_t = nc.dram_tensor("w_qk_t", (C, 2 * C), BF, kind="ExternalInput")
    w_v_t = nc.dram_tensor("w_v_t", (C, C), BF, kind="ExternalInput")
    w_qk_s = nc.dram_tensor("w_qk_s", (C, 2 * C), BF, kind="ExternalInput")
    w_v_s = nc.dram_tensor("w_v_s", (C, C), BF, kind="ExternalInput")
    w_ptfc = nc.dram_tensor("w_ptfc", (C, C), BF, kind="ExternalInput")
    w_proj_s = nc.dram_tensor("w_proj_s", (C, C), BF, kind="ExternalInput")
    w_fc1 = nc.dram_tensor("w_fc1", (C, HID), BF, kind="ExternalInput")
    w_fc2 = nc.dram_tensor("w_fc2", (HID, C), BF, kind="ExternalInput")
    sel12 = nc.dram_tensor("sel12", (12, C), BF, kind="ExternalInput")
    one8 = nc.dram_tensor("one8", (8, 1), BF, kind="ExternalInput")
    m01_in = nc.dram_tensor("m01", (P, 2, P), BF, kind="ExternalInput")
    ident_in = nc.dram_tensor("ident", (P, P), BF, kind="ExternalInput")
    out = nc.dram_tensor("out", (N, C), F32, kind="ExternalOutput")
    dbg = {}
    if KDEBUG:
        for nm, shp in (("d_qk_t", (P, 12, NT)), ("d_v_t", (P, 13, 12, D + 1)),
                        ("d_oT_t", (P, CO, NT)), ("d_xs", (8, NSEQ, C)),
                        ("d_qk_s", (P, 12, NS)),
                        ("d_v_s", (P, 8, 2, 12, D + 1)),
                        ("d_oT_s", (P, CO, NS)), ("d_y_s", (8, NSEQ, C)),
                        ("d_cls", (1, C))):
            dbg[nm] = nc.dram_tensor(nm, shp, BF, kind="ExternalOutput")

    with tile.TileContext(nc) as tc:
        with ExitStack() as root:
            dram = root.enter_context(tc.tile_pool(name="dram", bufs=1,
                                                   space="DRAM"))
            const = root.enter_context(tc.tile_pool(name="const", bufs=1))
            xs_d = dram.tile([8, NSEQ, C], BF)
            y_s = dram.tile([8, NSEQ, C], BF)
            cls_row = dram.tile([1, C], BF)

            ident = const.tile([P, P], BF)
            nc.sync.dma_start(ident, ident_in[:])
            m01 = const.tile([P, 2, P], BF)
            nc.sync.dma_start(m01, m01_in[:])
            eps_t = const.tile([P, 1], F32)
            nc.vector.memset(eps_t, EPS)
            sel_sb = const.tile([12, C], BF)
            nc.sync.dma_start(sel_sb, sel12[:])

            # Stack allocator: a pool reserves its full size at open, so
            # lifetimes must nest (LIFO).  Onion order, outermost first;
            # tiles/DMAs are issued later at the prefetch point.
            sWL = ExitStack()   # wfc1                [t0 .. end]
            wL = sWL.enter_context(tc.tile_pool(name="wL", bufs=1))
            sWC = ExitStack()   # wqk_s, wv_s         [t0 .. ph6]
            wC = sWC.enter_context(tc.tile_pool(name="wC", bufs=1))
            sWD = ExitStack()   # wproj_s             [t0 .. ph6]
            wD = sWD.enter_context(tc.tile_pool(name="wD", bufs=1))

            # ---- temporal phases 1-3 ----
            sPT = ExitStack()   # qk_t, v_t           [ph1 .. ph3]
            poolT = sPT.enter_context(tc.tile_pool(name="poolT", bufs=1))
            qk_t = poolT.tile([P, 12, NT], BF)
            v_t = poolT.tile([P, 13, 12, D + 1], BF)
            nc.gpsimd.memset(v_t[:, :, :, D:D + 1], 1.0)

            sWA = ExitStack()   # wqk_t, wv_t         [ph1]
            wA = sWA.enter_context(tc.tile_pool(name="wA", bufs=1))
            wqk_t = wA.tile([P, CO, 2 * C], BF)
            wv_t = wA.tile([P, CO, C], BF)

            def load_wA():
                nc.sync.dma_start(wqk_t,
                                  w_qk_t[:].rearrange("(co p) o -> p co o", p=P))
                nc.sync.dma_start(wv_t,
                                  w_v_t[:].rearrange("(co p) o -> p co o", p=P))

            if "1" in PHASES:
                with ExitStack() as ctx:
                    def src_t(x_t, i, rows):
                        nc.sync.dma_start(x_t[:rows],
                                          x_in[1 + i * P:1 + i * P + rows, :])
                    vchunks = [
                        ((lambda st: (lambda chlen, half:
                            v_t[0:chlen, st, half * 8:half * 8 + (8, 4)[half],
                                0:D]))(st),
                         st * P, min(P, NT - st * P))
                        for st in range(13)]
                    qkv_phase(nc, tc, ctx, src_t, NT, wqk_t, wv_t,
                              qk_t, v_t, vchunks, ident, eps_t,
                              wload=load_wA)
            sWA.close()

            sWB = ExitStack()   # wproj_t, wtfc       [ph2 .. ph3]
            wB = sWB.enter_context(tc.tile_pool(name="wB", bufs=1))
            wptfc = wB.tile([P, CO, C], BF)
            nc.sync.dma_start(wptfc,
                                w_ptfc[:].rearrange("(co p) o -> p co o", p=P))
            sOT = ExitStack()   # oT_t                [ph2 .. ph3]
            otp_ = sOT.enter_context(tc.tile_pool(name="oT_t_pool", bufs=1))
            oT_t = otp_.tile([P, CO, NT], BF)

            # prefetch phase-4 weights during phases 2-3
            wqk_s = wC.tile([P, CO, 2 * C], BF)
            nc.sync.dma_start(wqk_s, w_qk_s[:].rearrange("(co p) o -> p co o", p=P))
            wv_s = wC.tile([P, CO, C], BF)
            nc.sync.dma_start(wv_s, w_v_s[:].rearrange("(co p) o -> p co o", p=P))

            if "2" in PHASES:
                phase2_temporal_attn(nc, tc, qk_t, v_t, m01, sel_sb, oT_t)
                if KDEBUG:
                    nc.sync.dma_start(dbg["d_qk_t"][:], qk_t)
                    nc.sync.dma_start(dbg["d_v_t"][:], v_t)
                    nc.sync.dma_start(dbg["d_oT_t"][:], oT_t)
            if "3" in PHASES:
                phase3_temporal_proj(nc, tc, x_in, wptfc, oT_t, xs_d)
            sOT.close()
            sWB.close()
            sPT.close()

            # ---- spatial phases 4-6 ----
            wproj_s = wD.tile([P, CO, C], BF)
            nc.sync.dma_start(wproj_s,
                                w_proj_s[:].rearrange("(co p) o -> p co o", p=P))
            wfc1 = wL.tile([P, CO, HID], BF)
            nc.sync.dma_start(wfc1, w_fc1[:].rearrange("(co p) o -> p co o", p=P))

            sPS = ExitStack()   # qk_s, v_s, oT_s     [ph4 .. ph6]
            poolS = sPS.enter_context(tc.tile_pool(name="poolS", bufs=1))
            qk_s = poolS.tile([P, 12, NS], BF)
            v_s = poolS.tile([P, 8, 2, 12, D + 1], BF)
            nc.gpsimd.memset(v_s[:, :, :, :, D:D + 1], 1.0)
            oT_s = poolS.tile([P, CO, NS], BF)

            if "4" in PHASES:
                with ExitStack() as ctx:
                    xs_flat = xs_d[:].rearrange("f n c -> (f n) c")

                    def src_s(x_t, i, rows):
                        nc.sync.dma_start(x_t[:rows],
                                          xs_flat[i * P:i * P + rows, :])
                    vchunks = [
                        ((lambda f, chi: (lambda chlen, half:
                            v_s[0:chlen, f, chi, half * 8:half * 8 + (8, 4)[half],
                                0:D]))(f, chi),
                         f * NSEQ + chi * P, (P, NSEQ - P)[chi])
                        for f in range(8) for chi in range(2)]
                    qkv_phase(nc, tc, ctx, src_s, NS, wqk_s, wv_s,
                              qk_s, v_s, vchunks, ident, eps_t)
            if "5" in PHASES:
                phase5_spatial_attn(nc, tc, qk_s, v_s, sel_sb, oT_s)
                if KDEBUG:
                    nc.sync.dma_start(dbg["d_qk_s"][:], qk_s)
                    nc.sync.dma_start(dbg["d_v_s"][:], v_s)
                    nc.sync.dma_start(dbg["d_oT_s"][:], oT_s)
            if "6" in PHASES:
                phase6_spatial_proj(nc, tc, xs_d, wproj_s, oT_s, y_s)
            sPS.close()
            sWD.close()
            sWC.close()

            # ---- cls + MLP ----
            sWF = ExitStack()   # wfc2                [ph7 .. end]
            wF = sWF.enter_context(tc.tile_pool(name="wF", bufs=1))
            wfc2 = wF.tile([P, HIDO, C], BF)

            def load_wF():
                nc.sync.dma_start(wfc2,
                                  w_fc2[:].rearrange("(ho p) o -> p ho o", p=P))

            if "7" in PHASES:
                phase7_cls(nc, tc, y_s, one8, cls_row)
            if "8" in PHASES:
                phase8_mlp(nc, tc, y_s, cls_row, wfc1, wfc2, out, ident, eps_t,
                           wload=load_wF)
            elif True:
                load_wF()
            sWF.close()
            sWL.close()

            if KDEBUG:
                nc.sync.dma_start(dbg["d_xs"][:], xs_d[:])
                nc.sync.dma_start(dbg["d_y_s"][:], y_s[:])
                nc.sync.dma_start(dbg["d_cls"][:], cls_row[:])

    nc.compile()
    return nc


_NC_CACHE = None


def _get_nc():
    global _NC_CACHE
    if _NC_CACHE is None:
        _NC_CACHE = build_nc()
    return _NC_CACHE


def make_consts():
    bf = ml_dtypes.bfloat16
    sel = np.zeros((12, C), np.float32)
    for pr in range(6):
        for p in range(P):
            sel[2 * pr + p // D, pr * P + p] = 1.0
    one8 = np.full((8, 1), 0.125, np.float32)
    m01 = np.zeros((P, P), np.float32)
    for s in range(16):
        m01[s * 8:(s + 1) * 8, s * 8:(s + 1) * 8] = 1.0
    m01 = np.repeat(m01[:, None, :], 2, axis=1)
    ident = np.eye(P, dtype=np.float32)
    return sel.astype(bf), one8.astype(bf), m01.astype(bf), ident.astype(bf)


def kernel(**inputs):
    bf = ml_dtypes.bfloat16
    x = np.asarray(inputs["x"], dtype=np.float32)
    B = x.shape[0]
    tr = lambda w: np.ascontiguousarray(
        np.asarray(w, np.float32).T.astype(bf))
    qkv_w = np.asarray(inputs["qkv_w"], np.float32)
    tqkv_w = np.asarray(inputs["tqkv_w"], np.float32)
    sel, one8, m01, ident = make_consts()
    shared = {
        "w_qk_t": tr(tqkv_w[:2 * C]), "w_v_t": tr(tqkv_w[2 * C:]),
        "w_qk_s": tr(qkv_w[:2 * C]), "w_v_s": tr(qkv_w[2 * C:]),
        "w_ptfc": tr(np.asarray(inputs["tfc_w"], np.float32)
                     @ np.asarray(inputs["tproj_w"], np.float32)),
        "w_proj_s": tr(inputs["proj_w"]),
        "w_fc1": tr(inputs["fc1_w"]), "w_fc2": tr(inputs["fc2_w"]),
        "sel12": sel, "one8": one8, "m01": m01, "ident": ident,
    }
    nc = _get_nc()
    in_maps = [dict(shared, x=np.ascontiguousarray(x[b].astype(bf)))
               for b in range(B)]
    res = run_bass_kernel_spmd(nc, in_maps, core_ids=list(range(B)),
                               trace=bool(int(os.environ.get("KTRACE", "0"))))
    out = np.stack([res.results[b]["out"] for b in range(B)], axis=0)
    kernel.last_results = res
    return out


# revision 25
# speedup vs baseline: 1.7641x; 1.0020x over previous
"""TimeSformer-style block (temporal attn -> spatial attn -> MLP) on 8 trn2 cores.

Data-parallel over B=8: each NeuronCore processes one batch element end to end.
All GEMMs run in bf16 (1 cycle/row on the PE at any free-dim size, 4x cheaper
weight loads than fp32r); PSUM accumulation stays fp32. Attention q/k/v images
live in SBUF (no HBM round trips); the MLP is fused per 512-token block so the
fc1 activation image never touches DRAM. Block-diagonal attention masking uses
a multiplicative 0/1 mask after exp instead of extra contraction rows.
"""

import os
import sys
from contextlib import ExitStack

sys.path.insert(0, "/opt/trn_rl_repo")

import numpy as np
import ml_dtypes

import concourse.bass as bass
import concourse.mybir as mybir
import concourse.tile as tile
from concourse import bacc
from concourse.bass_utils import run_bass_kernel_spmd

F32 = mybir.dt.float32
BF = mybir.dt.bfloat16
AF = mybir.ActivationFunctionType
ALU = mybir.AluOpType

C = 768
CO = 6           # C / 128
H = 12
D = 64
T = 8
G = 196          # h*w sequences
NT = G * T       # 1568 temporal tokens
NSEQ = 197       # spatial seq len (cls + 196)
NS = 8 * NSEQ    # 1576 spatial tokens
N = 1569
HID = 3072
HIDO = 24        # HID / 128
P = 128
EPS = 1e-5
SCALE = D ** -0.5

PHASES = os.environ.get("KPHASES", "12345678")
KDEBUG = bool(int(os.environ.get("KDEBUG", "0")))


def _ceil(a, b):
    return (a + b - 1) // b


def ln_tile(nc, tmp, out_pool, x_t, rows, eps_t):
    """LayerNorm over free dim (768) of a [rows<=128, 768] token-major tile."""
    stats = tmp.tile([P, 3, 6], F32, tag="ln_stats")
    for s in range(3):
        nc.vector.bn_stats(out=stats[:rows, s], in_=x_t[:rows, s * 256:(s + 1) * 256])
    mv = tmp.tile([P, 2], F32, tag="ln_mv")
    nc.vector.bn_aggr(out=mv[:rows], in_=stats[:rows])
    nc.scalar.activation(out=mv[:rows, 1:2], in_=mv[:rows, 1:2], func=AF.Sqrt,
                         bias=eps_t[:rows], scale=1.0)
    nc.vector.reciprocal(out=mv[:rows, 1:2], in_=mv[:rows, 1:2])
    xln = out_pool.tile([P, C], BF, tag="ln_out")
    nc.vector.tensor_scalar(out=xln[:rows], in0=x_t[:rows],
                            scalar1=mv[:rows, 0:1], scalar2=mv[:rows, 1:2],
                            op0=ALU.subtract, op1=ALU.mult)
    return xln


def qkv_phase(nc, tc, ctx, src_dma, ntok, wqk, wv, qk_img, v_img, vchunks,
              ident, eps_t, wload=None):
    """LN -> PE transpose -> qk (c-major SBUF image) + v (token-major chunks)."""
    lnp = ctx.enter_context(tc.tile_pool(name="lnp", bufs=3))
    lnout = ctx.enter_context(tc.tile_pool(name="lnout", bufs=3))
    tpp = ctx.enter_context(tc.tile_pool(name="tp_ps", bufs=3, space="PSUM"))
    xlp = ctx.enter_context(tc.tile_pool(name="xlnT", bufs=1))
    mmp = ctx.enter_context(tc.tile_pool(name="mm_ps", bufs=5, space="PSUM"))

    xlnT = xlp.tile([P, CO, ntok], BF)
    ntiles = _ceil(ntok, P)
    for i in range(ntiles):
        rows = min(P, ntok - i * P)
        x_t = lnp.tile([P, C], BF, tag="x_t")
        src_dma(x_t, i, rows)
        xln = ln_tile(nc, lnp, lnout, x_t, rows, eps_t)
        for cp in range(CO // 2):
            pt = tpp.tile([P, 2, P], BF, tag="tp")
            for k in range(2):
                co = 2 * cp + k
                nc.tensor.transpose(pt[:, k, :rows],
                                    xln[:rows, co * P:(co + 1) * P],
                                    ident[:rows, :rows])
            nc.vector.tensor_copy(
                out=xlnT[:, 2 * cp:2 * cp + 2, i * P:i * P + rows],
                in_=pt[:, :, :rows])
        if i == 0 and wload is not None:
            wload()
    for b in range(_ceil(ntok, 512)):
        cols = min(512, ntok - b * 512)
        for o in range(12):  # 2C/128 output chunks (q then k)
            ps = mmp.tile([P, 512], F32, tag="mm")
            for co in range(CO):
                nc.tensor.matmul(ps[:, :cols], wqk[:, co, o * P:(o + 1) * P],
                                 xlnT[:, co, b * 512:b * 512 + cols],
                                 start=(co == 0), stop=(co == CO - 1))
            nc.scalar.copy(out=qk_img[:, o, b * 512:b * 512 + cols],
                           in_=ps[:, :cols])
    for dst_fn, col0v, chlen in vchunks:
        for half, fcols in ((0, 512), (1, 256)):
            ps = mmp.tile([P, 512], F32, tag="mm")
            for co in range(CO):
                nc.tensor.matmul(ps[:chlen, :fcols],
                                 xlnT[:, co, col0v:col0v + chlen],
                                 wv[:, co, half * 512:half * 512 + fcols],
                                 start=(co == 0), stop=(co == CO - 1))
            nc.vector.tensor_copy(out=dst_fn(chlen, half), in_=ps[:chlen, :fcols])


def phase2_temporal_attn(nc, tc, qk_t, v_t, m01_2, sel_sb, oT_t):
    """Subtile-outer with lag-1 AV issue and per-subtile normalize; every
    matmul writes its PSUM tile at offset 0 (free-offset accumulation hangs
    the device)."""
    with ExitStack() as ctx:
        sp = ctx.enter_context(tc.tile_pool(name="t_sps", bufs=4, space="PSUM"))
        op = ctx.enter_context(tc.tile_pool(name="t_ops", bufs=2, space="PSUM"))
        pp = ctx.enter_context(tc.tile_pool(name="t_p", bufs=8))
        sig = ctx.enter_context(tc.tile_pool(name="t_sig", bufs=1))
        bcp = ctx.enter_context(tc.tile_pool(name="t_bc", bufs=2, space="PSUM"))

        sigma = sig.tile([12, NT], F32)
        rinv = sig.tile([12, NT], BF)
        nst = _ceil(NT, P)

        def attn_head(st, h, rows, pt):
            po = op.tile([D + 1, P], F32, tag="o_ps")
            nc.tensor.matmul(po[:, :rows], v_t[0:rows, st, h, 0:D + 1],
                             pt[:rows, :rows], start=True, stop=True)
            hp, hc = (h % 2) * D, h // 2
            nc.vector.tensor_copy(out=oT_t[hp:hp + D, hc, st * P:st * P + rows],
                                  in_=po[0:D, :rows])
            sgst = pp.tile([1, P], F32, tag="sg_st")
            nc.scalar.copy(out=sgst, in_=po[D:D + 1])
            nc.sync.dma_start(sigma[h:h + 1, st * P:st * P + rows],
                              sgst[0:1, :rows])

        def normalize(st, rows):
            with nc.allow_low_precision(reason="rinv bf16 feeds bcast matmul"):
                nc.vector.reciprocal(out=rinv[:, st * P:st * P + rows],
                                     in_=sigma[:, st * P:st * P + rows])
            for pr in range(CO):
                bc = bcp.tile([P, P], F32, tag="bc_ps")
                nc.tensor.matmul(bc[:, :rows], sel_sb[:, pr * P:(pr + 1) * P],
                                 rinv[:, st * P:st * P + rows],
                                 start=True, stop=True)
                nc.vector.tensor_mul(out=oT_t[:, pr, st * P:st * P + rows],
                                     in0=oT_t[:, pr, st * P:st * P + rows],
                                     in1=bc[:, :rows])

        pend = None
        for st in range(nst):
            rows = min(P, NT - st * P)
            for h in range(H):
                hp, hc = (h % 2) * D, h // 2
                ps = sp.tile([P, P], F32, tag="s_ps")
                nc.tensor.matmul(ps[:rows, :rows],
                                 qk_t[hp:hp + D, 6 + hc, st * P:st * P + rows],
                                 qk_t[hp:hp + D, hc, st * P:st * P + rows],
                                 start=True, stop=True)
                pe = pp.tile([P, P], BF, tag="p_e")
                nc.scalar.activation(out=pe[:rows], in_=ps[:rows],
                                     func=AF.Exp, scale=SCALE)
                pt = pp.tile([P, P], BF, tag="p_t")
                eng = nc.gpsimd if h % 2 else nc.vector
                eng.tensor_mul(out=pt[:rows], in0=pe[:rows],
                               in1=m01_2[:rows, 0])
                if pend is not None:
                    attn_head(*pend)
                pend = (st, h, rows, pt)
            if st > 0:
                normalize(st - 1, P)
        attn_head(*pend)
        normalize(nst - 1, min(P, NT - (nst - 1) * P))


def phase3_temporal_proj(nc, tc, x_in, wptfc, oT_t, xs_d):
    """xt = x + oT @ (tfc_w @ tproj_w).T, scattered to spatial layout."""
    with ExitStack() as ctx:
        mp = ctx.enter_context(tc.tile_pool(name="p3_ps", bufs=4, space="PSUM"))
        tp = ctx.enter_context(tc.tile_pool(name="p3_t", bufs=4))

        # xs_d[f, 0, :] = x[0] (cls) for every frame
        cls_sb = tp.tile([8, C], BF, tag="cls_sb")
        nc.gpsimd.dma_start(cls_sb, bass.AP(tensor=x_in, offset=0,
                                            ap=[[0, 8], [1, C]]))
        nc.sync.dma_start(xs_d[:, 0, :], cls_sb)

        for i in range(_ceil(NT, P)):
            tok0 = i * P
            rows = min(P, NT - tok0)
            x_t = tp.tile([P, C], BF, tag="x_t3")
            nc.sync.dma_start(x_t[:rows], x_in[1 + tok0:1 + tok0 + rows, :])
            xt = tp.tile([P, C], BF, tag="xt3")
            for half, fcols in ((0, 512), (1, 256)):
                ps = mp.tile([P, 512], F32, tag="p3ps")
                for co in range(CO):
                    nc.tensor.matmul(
                        ps[:rows, :fcols], oT_t[:, co, tok0:tok0 + rows],
                        wptfc[:, co, half * 512:half * 512 + fcols],
                        start=(co == 0), stop=(co == CO - 1))
                nc.vector.tensor_add(
                    out=xt[:rows, half * 512:half * 512 + fcols],
                    in0=ps[:rows, :fcols],
                    in1=x_t[:rows, half * 512:half * 512 + fcols])
            # token g*8+t -> xs_d[t, 1+g]; permutation on the DRAM-side AP
            g0 = tok0 // T
            ng = rows // T
            nc.sync.dma_start(
                xs_d[:, 1 + g0:1 + g0 + ng, :].rearrange("t g c -> g t c"),
                xt[:rows])


def phase5_spatial_attn(nc, tc, qk_s, v_s, sel_sb, oT_s):
    """Per-frame attention with lag-1 AV issue and frame-lagged normalize.
    All PSUM matmul outputs at tile offset 0."""
    with ExitStack() as ctx:
        sp = ctx.enter_context(tc.tile_pool(name="s_sps", bufs=4, space="PSUM"))
        op = ctx.enter_context(tc.tile_pool(name="s_ops", bufs=2, space="PSUM"))
        pp = ctx.enter_context(tc.tile_pool(name="s_p", bufs=6))
        sig = ctx.enter_context(tc.tile_pool(name="s_sig", bufs=2))
        bcp = ctx.enter_context(tc.tile_pool(name="s_bc", bufs=2, space="PSUM"))

        CHUNKS = ((0, 0, P), (1, P, NSEQ - P))

        def do_av(f, h, pts, sigma):
            hp, hc = (h % 2) * D, h // 2
            col0 = f * NSEQ
            po = op.tile([D + 1, NSEQ], F32, tag="o_ps_s")
            for ch, off, chlen in CHUNKS:
                nc.tensor.matmul(po, v_s[0:chlen, f, ch, h, 0:D + 1],
                                 pts[ch][:chlen], start=(ch == 0),
                                 stop=(ch == 1))
            nc.vector.tensor_copy(out=oT_s[hp:hp + D, hc, col0:col0 + NSEQ],
                                  in_=po[0:D])
            sgst = pp.tile([1, NSEQ], F32, tag="sg_st")
            nc.scalar.copy(out=sgst, in_=po[D:D + 1])
            nc.sync.dma_start(sigma[h:h + 1], sgst)

        def normalize(f, sigma, rinv):
            col0 = f * NSEQ
            with nc.allow_low_precision(reason="rinv bf16 feeds bcast matmul"):
                nc.vector.reciprocal(out=rinv, in_=sigma)
            for pr in range(CO):
                bc = bcp.tile([P, NSEQ], F32, tag="bc_s")
                nc.tensor.matmul(bc, sel_sb[:, pr * P:(pr + 1) * P], rinv,
                                 start=True, stop=True)
                nc.vector.tensor_mul(out=oT_s[:, pr, col0:col0 + NSEQ],
                                     in0=oT_s[:, pr, col0:col0 + NSEQ], in1=bc)

        pend = None
        prev_sig = None
        for f in range(8):
            col0 = f * NSEQ
            sigma = sig.tile([12, NSEQ], F32, tag="sig_s")
            rinv = sig.tile([12, NSEQ], BF, tag="rinv_s")
            for h in range(H):
                hp, hc = (h % 2) * D, h // 2
                pts = []
                for ch, off, chlen in CHUNKS:
                    ps = sp.tile([P, NSEQ], F32, tag="s_ps_s")
                    nc.tensor.matmul(
                        ps[:chlen],
                        qk_s[hp:hp + D, 6 + hc, col0 + off:col0 + off + chlen],
                        qk_s[hp:hp + D, hc, col0:col0 + NSEQ],
                        start=True, stop=True)
                    pt = pp.tile([P, NSEQ], BF, tag="p_s")
                    nc.scalar.activation(out=pt[:chlen], in_=ps[:chlen],
                                         func=AF.Exp, scale=SCALE)
                    pts.append(pt)
                if pend is not None:
                    do_av(*pend)
                pend = (f, h, pts, sigma)
            if prev_sig is not None:
                normalize(f - 1, *prev_sig)
            prev_sig = (sigma, rinv)
        do_av(*pend)
        normalize(7, *prev_sig)


def phase6_spatial_proj(nc, tc, xs_d, wproj_s, oT_s, y_s):
    with ExitStack() as ctx:
        mp = ctx.enter_context(tc.tile_pool(name="p6_ps", bufs=4, space="PSUM"))
        tp = ctx.enter_context(tc.tile_pool(name="p6_t", bufs=4))
        xs_flat = xs_d[:].rearrange("f n c -> (f n) c")
        ys_flat = y_s[:].rearrange("f n c -> (f n) c")
        for i in range(_ceil(NS, P)):
            rows = min(P, NS - i * P)
            x_t = tp.tile([P, C], BF, tag="x_t6")
            nc.sync.dma_start(x_t[:rows], xs_flat[i * P:i * P + rows, :])
            yt = tp.tile([P, C], BF, tag="yt6")
            for half, fcols in ((0, 512), (1, 256)):
                ps = mp.tile([P, 512], F32, tag="p6ps")
                for co in range(CO):
                    nc.tensor.matmul(
                        ps[:rows, :fcols], oT_s[:, co, i * P:i * P + rows],
                        wproj_s[:, co, half * 512:half * 512 + fcols],
                        start=(co == 0), stop=(co == CO - 1))
                nc.vector.tensor_add(
                    out=yt[:rows, half * 512:half * 512 + fcols],
                    in0=ps[:rows, :fcols],
                    in1=x_t[:rows, half * 512:half * 512 + fcols])
            nc.sync.dma_start(ys_flat[i * P:i * P + rows, :], yt[:rows])


def phase7_cls(nc, tc, y_s, one8_in, cls_row):
    with ExitStack() as ctx:
        tp = ctx.enter_context(tc.tile_pool(name="p7_t", bufs=1))
        mp = ctx.enter_context(tc.tile_pool(name="p7_ps", bufs=2, space="PSUM"))
        ycls = tp.tile([8, C], BF)
        nc.sync.dma_start(ycls, y_s[:, 0, :])
        o8 = tp.tile([8, 1], BF)
        nc.sync.dma_start(o8, one8_in[:])
        res = tp.tile([1, C], BF)
        for half, fcols in ((0, 512), (1, 256)):
            ps = mp.tile([1, 512], F32, tag="p7_ps")
            nc.tensor.matmul(ps[:, :fcols], o8,
                             ycls[:, half * 512:half * 512 + fcols],
                             start=True, stop=True)
            nc.vector.tensor_copy(out=res[:, half * 512:half * 512 + fcols],
                                  in_=ps[:, :fcols])
        nc.sync.dma_start(cls_row[:], res)


def phase8_mlp(nc, tc, y_s, cls_row, wfc1, wfc2, out, ident, eps_t,
               wload=None):
    blocks = [(0, 512), (512, 512), (1024, 512), (1536, 33)]
    with ExitStack() as ctx:
        lnp = ctx.enter_context(tc.tile_pool(name="p8_ln", bufs=3))
        lnout = ctx.enter_context(tc.tile_pool(name="p8_lno", bufs=3))
        xrp = ctx.enter_context(tc.tile_pool(name="p8_xr", bufs=5))
        tpp = ctx.enter_context(tc.tile_pool(name="p8_tp", bufs=3, space="PSUM"))
        xlp = ctx.enter_context(tc.tile_pool(name="p8_xlT", bufs=2))
        mp = ctx.enter_context(tc.tile_pool(name="p8_ps", bufs=5, space="PSUM"))
        h1p = ctx.enter_context(tc.tile_pool(name="p8_h1", bufs=2))
        otp = ctx.enter_context(tc.tile_pool(name="p8_o", bufs=3))

        for tok0, btok in blocks:
            nits = _ceil(btok, P)
            xlnT = xlp.tile([P, CO, 512], BF, tag="xlnT8")
            xts = []
            for it in range(nits):
                rows = min(P, btok - it * P)
                x_t = xrp.tile([P, C], BF, tag="x_t8")
                g0 = (tok0 + it * P) // T
                if btok == 33:
                    nc.sync.dma_start(
                        x_t[:32],
                        y_s[:, 1 + g0:1 + g0 + 4, :].rearrange("t g c -> g t c"))
                    nc.sync.dma_start(x_t[32:33], cls_row[:])
                else:
                    nc.sync.dma_start(
                        x_t[:rows],
                        y_s[:, 1 + g0:1 + g0 + 16, :].rearrange("t g c -> g t c"))
                xts.append((x_t, rows))
                if wload is not None:
                    wload()
                    wload = None
                xln = ln_tile(nc, lnp, lnout, x_t, rows, eps_t)
                for cp in range(CO // 2):
                    pt = tpp.tile([P, 2, P], BF, tag="tp8")
                    for k in range(2):
                        co = 2 * cp + k
                        nc.tensor.transpose(pt[:, k, :rows],
                                            xln[:rows, co * P:(co + 1) * P],
                                            ident[:rows, :rows])
                    nc.vector.tensor_copy(
                        out=xlnT[:, 2 * cp:2 * cp + 2, it * P:it * P + rows],
                        in_=pt[:, :, :rows])
            h1 = h1p.tile([P, HIDO, 512], BF, tag="h1blk")
            for o in range(HIDO):
                ps = mp.tile([P, 512], F32, tag="p8ps")
                for co in range(CO):
                    nc.tensor.matmul(ps[:, :btok], wfc1[:, co, o * P:(o + 1) * P],
                                     xlnT[:, co, :btok],
                                     start=(co == 0), stop=(co == CO - 1))
                nc.scalar.activation(out=h1[:, o, :btok], in_=ps[:, :btok],
                                     func=AF.Gelu)
            for it in range(nits):
                x_t, rows = xts[it]
                o_t = otp.tile([P, C], F32, tag="o_t8")
                for half, fcols in ((0, 512), (1, 256)):
                    ps = mp.tile([P, 512], F32, tag="p8ps")
                    for ho in range(HIDO):
                        nc.tensor.matmul(
                            ps[:rows, :fcols], h1[:, ho, it * P:it * P + rows],
                            wfc2[:, ho, half * 512:half * 512 + fcols],
                            start=(ho == 0), stop=(ho == HIDO - 1))
                    nc.vector.tensor_add(
                        out=o_t[:rows, half * 512:half * 512 + fcols],
                        in0=ps[:rows, :fcols],
                        in1=x_t[:rows, half * 512:half * 512 + fcols])
                row0 = tok0 + it * P
                if btok == 33:
                    nc.sync.dma_start(out[1 + row0:1 + row0 + 32, :], o_t[:32])
                    nc.sync.dma_start(out[0:1, :], o_t[32:33])
                else:
                    nc.sync.dma_start(out[1 + row0:1 + row0 + rows, :],
                                      o_t[:rows])


def build_nc():
    nc = bacc.Bacc("TRN2", target_bir_lowering=False, debug=False)

    x_in = nc.dram_tensor("x", (N, C), BF, kind="ExternalInput")
    w_qk_t = nc.dram_tensor("w_qk_t", (C, 2 * C), BF, kind="ExternalInput")
    w_v_t = nc.dram_tensor("w_v_t", (C, C), BF, kind="ExternalInput")
    w_qk_s = nc.dram_tensor("w_qk_s", (C, 2 * C), BF, kind="ExternalInput")
    w_v_s = nc.dram_tensor("w_v_s", (C, C), BF, kind="ExternalInput")
    w_ptfc = nc.dram_tensor("w_ptfc", (C, C), BF, kind="ExternalInput")
    w_proj_s = nc.dram_tensor("w_proj_s", (C, C), BF, kind="ExternalInput")
    w_fc1 = nc.dram_tensor("w_fc1", (C, HID), BF, kind="ExternalInput")
    w_fc2 = nc.dram_tensor("w_fc2", (HID, C), BF, kind="ExternalInput")
    sel12 = nc.dram_tensor("sel12", (12, C), BF, kind="ExternalInput")
    one8 = nc.dram_tensor("one8", (8, 1), BF, kind="ExternalInput")
    m01_in = nc.dram_tensor("m01", (P, 2, P), BF, kind="ExternalInput")
    ident_in = nc.dram_tensor("ident", (P, P), BF, kind="ExternalInput")
    out = nc.dram_tensor("out", (N, C), F32, kind="ExternalOutput")
    dbg = {}
    if KDEBUG:
        for nm, shp in (("d_qk_t", (P, 12, NT)), ("d_v_t", (P, 13, 12, D + 1)),
                        ("d_oT_t", (P, CO, NT)), ("d_xs", (8, NSEQ, C)),
                        ("d_qk_s", (P, 12, NS)),
                        ("d_v_s", (P, 8, 2, 12, D + 1)),
                        ("d_oT_s", (P, CO, NS)), ("d_y_s", (8, NSEQ, C)),
                        ("d_cls", (1, C))):
            dbg[nm] = nc.dram_tensor(nm, shp, BF, kind="ExternalOutput")

    with tile.TileContext(nc) as tc:
        with ExitStack() as root:
            dram = root.enter_context(tc.tile_pool(name="dram", bufs=1,
                                                   space="DRAM"))
            const = root.enter_context(tc.tile_pool(name="const", bufs=1))
            xs_d = dram.tile([8, NSEQ, C], BF)
            y_s = dram.tile([8, NSEQ, C], BF)
            cls_row = dram.tile([1, C], BF)

            ident = const.tile([P, P], BF)
            nc.sync.dma_start(ident, ident_in[:])
            m01 = const.tile([P, 2, P], BF)
            nc.sync.dma_start(m01, m01_in[:])
            eps_t = const.tile([P, 1], F32)
            nc.vector.memset(eps_t, EPS)
            sel_sb = const.tile([12, C], BF)
            nc.sync.dma_start(sel_sb, sel12[:])

            # Stack allocator: a pool reserves its full size at open, so
            # lifetimes must nest (LIFO).  Onion order, outermost first;
            # tiles/DMAs are issued later at the prefetch point.
            sWL = ExitStack()   # wfc1                [t0 .. end]
            wL = sWL.enter_context(tc.tile_pool(name="wL", bufs=1))
            sWC = ExitStack()   # wqk_s, wv_s         [t0 .. ph6]
            wC = sWC.enter_context(tc.tile_pool(name="wC", bufs=1))
            sWD = ExitStack()   # wproj_s             [t0 .. ph6]
            wD = sWD.enter_context(tc.tile_pool(name="wD", bufs=1))

            # ---- temporal phases 1-3 ----
            sPT = ExitStack()   # qk_t, v_t           [ph1 .. ph3]
            poolT = sPT.enter_context(tc.tile_pool(name="poolT", bufs=1))
            qk_t = poolT.tile([P, 12, NT], BF)
            v_t = poolT.tile([P, 13, 12, D + 1], BF)
            nc.gpsimd.memset(v_t[:, :, :, D:D + 1], 1.0)

            sWA = ExitStack()   # wqk_t, wv_t         [ph1]
            wA = sWA.enter_context(tc.tile_pool(name="wA", bufs=1))
            wqk_t = wA.tile([P, CO, 2 * C], BF)
            wv_t = wA.tile([P, CO, C], BF)

            def load_wA():
                nc.sync.dma_start(wqk_t,
                                  w_qk_t[:].rearrange("(co p) o -> p co o", p=P))
                nc.sync.dma_start(wv_t,
                                  w_v_t[:].rearrange("(co p) o -> p co o", p=P))

            if "1" in PHASES:
                with ExitStack() as ctx:
                    def src_t(x_t, i, rows):
                        nc.sync.dma_start(x_t[:rows],
                                          x_in[1 + i * P:1 + i * P + rows, :])
                    vchunks = [
                        ((lambda st: (lambda chlen, half:
                            v_t[0:chlen, st, half * 8:half * 8 + (8, 4)[half],
                                0:D]))(st),
                         st * P, min(P, NT - st * P))
                        for st in range(13)]
                    qkv_phase(nc, tc, ctx, src_t, NT, wqk_t, wv_t,
                              qk_t, v_t, vchunks, ident, eps_t,
                              wload=load_wA)
            sWA.close()

            sWB = ExitStack()   # wproj_t, wtfc       [ph2 .. ph3]
            wB = sWB.enter_context(tc.tile_pool(name="wB", bufs=1))
            wptfc = wB.tile([P, CO, C], BF)
            nc.sync.dma_start(wptfc,
                                w_ptfc[:].rearrange("(co p) o -> p co o", p=P))
            sOT = ExitStack()   # oT_t                [ph2 .. ph3]
            otp_ = sOT.enter_context(tc.tile_pool(name="oT_t_pool", bufs=1))
            oT_t = otp_.tile([P, CO, NT], BF)

            # prefetch phase-4 weights during phases 2-3
            wqk_s = wC.tile([P, CO, 2 * C], BF)
            nc.sync.dma_start(wqk_s, w_qk_s[:].rearrange("(co p) o -> p co o", p=P))
            wv_s = wC.tile([P, CO, C], BF)
            nc.sync.dma_start(wv_s, w_v_s[:].rearrange("(co p) o -> p co o", p=P))

            if "2" in PHASES:
                phase2_temporal_attn(nc, tc, qk_t, v_t, m01, sel_sb, oT_t)
                if KDEBUG:
                    nc.sync.dma_start(dbg["d_qk_t"][:], qk_t)
                    nc.sync.dma_start(dbg["d_v_t"][:], v_t)
                    nc.sync.dma_start(dbg["d_oT_t"][:], oT_t)
            if "3" in PHASES:
                phase3_temporal_proj(nc, tc, x_in, wptfc, oT_t, xs_d)
            sOT.close()
            sWB.close()
            sPT.close()

            # ---- spatial phases 4-6 ----
            wproj_s = wD.tile([P, CO, C], BF)
            nc.sync.dma_start(wproj_s,
                                w_proj_s[:].rearrange("(co p) o -> p co o", p=P))
            wfc1 = wL.tile([P, CO, HID], BF)
            nc.sync.dma_start(wfc1, w_fc1[:].rearrange("(co p) o -> p co o", p=P))

            sPS = ExitStack()   # qk_s, v_s, oT_s     [ph4 .. ph6]
            poolS = sPS.enter_context(tc.tile_pool(name="poolS", bufs=1))
            qk_s = poolS.tile([P, 12, NS], BF)
            v_s = poolS.tile([P, 8, 2, 12, D + 1], BF)
            nc.gpsimd.memset(v_s[:, :, :, :, D:D + 1], 1.0)
            oT_s = poolS.tile([P, CO, NS], BF)

            if "4" in PHASES:
                with ExitStack() as ctx:
                    xs_flat = xs_d[:].rearrange("f n c -> (f n) c")

                    def src_s(x_t, i, rows):
                        nc.sync.dma_start(x_t[:rows],
                                          xs_flat[i * P:i * P + rows, :])
                    vchunks = [
                        ((lambda f, chi: (lambda chlen, half:
                            v_s[0:chlen, f, chi, half * 8:half * 8 + (8, 4)[half],
                                0:D]))(f, chi),
                         f * NSEQ + chi * P, (P, NSEQ - P)[chi])
                        for f in range(8) for chi in range(2)]
                    qkv_phase(nc, tc, ctx, src_s, NS, wqk_s, wv_s,
                              qk_s, v_s, vchunks, ident, eps_t)
            if "5" in PHASES:
                phase5_spatial_attn(nc, tc, qk_s, v_s, sel_sb, oT_s)
                if KDEBUG:
                    nc.sync.dma_start(dbg["d_qk_s"][:], qk_s)
                    nc.sync.dma_start(dbg["d_v_s"][:], v_s)
                    nc.sync.dma_start(dbg["d_oT_s"][:], oT_s)
            if "6" in PHASES:
                phase6_spatial_proj(nc, tc, xs_d, wproj_s, oT_s, y_s)
            sPS.close()
            sWD.close()
            sWC.close()

            # ---- cls + MLP ----
            sWF = ExitStack()   # wfc2                [ph7 .. end]
            wF = sWF.enter_context(tc.tile_pool(name="wF", bufs=1))
            wfc2 = wF.tile([P, HIDO, C], BF)

            def load_wF():
                nc.sync.dma_start(wfc2,
                                  w_fc2[:].rearrange("(ho p) o -> p ho o", p=P))

            if "7" in PHASES:
                phase7_cls(nc, tc, y_s, one8, cls_row)
            if "8" in PHASES:
                phase8_mlp(nc, tc, y_s, cls_row, wfc1, wfc2, out, ident, eps_t,
                           wload=load_wF)
            elif True:
                load_wF()
            sWF.close()
            sWL.close()

            if KDEBUG:
                nc.sync.dma_start(dbg["d_xs"][:], xs_d[:])
                nc.sync.dma_start(dbg["d_y_s"][:], y_s[:])
                nc.sync.dma_start(dbg["d_cls"][:], cls_row[:])

    nc.compile()
    return nc


_NC_CACHE = None


def _get_nc():
    global _NC_CACHE
    if _NC_CACHE is None:
        _NC_CACHE = build_nc()
    return _NC_CACHE


def make_consts():
    bf = ml_dtypes.bfloat16
    sel = np.zeros((12, C), np.float32)
    for pr in range(6):
        for p in range(P):
            sel[2 * pr + p // D, pr * P + p] = 1.0
    one8 = np.full((8, 1), 0.125, np.float32)
    m01 = np.zeros((P, P), np.float32)
    for s in range(16):
        m01[s * 8:(s + 1) * 8, s * 8:(s + 1) * 8] = 1.0
    m01 = np.repeat(m01[:, None, :], 2, axis=1)
    ident = np.eye(P, dtype=np.float32)
    return sel.astype(bf), one8.astype(bf), m01.astype(bf), ident.astype(bf)


def kernel(**inputs):
    bf = ml_dtypes.bfloat16
    x = np.asarray(inputs["x"], dtype=np.float32)
    B = x.shape[0]
    tr = lambda w: np.ascontiguousarray(
        np.asarray(w, np.float32).T.astype(bf))
    qkv_w = np.asarray(inputs["qkv_w"], np.float32)
    tqkv_w = np.asarray(inputs["tqkv_w"], np.float32)
    sel, one8, m01, ident = make_consts()
    shared = {
        "w_qk_t": tr(tqkv_w[:2 * C]), "w_v_t": tr(tqkv_w[2 * C:]),
        "w_qk_s": tr(qkv_w[:2 * C]), "w_v_s": tr(qkv_w[2 * C:]),
        "w_ptfc": tr(np.asarray(inputs["tfc_w"], np.float32)
                     @ np.asarray(inputs["tproj_w"], np.float32)),
        "w_proj_s": tr(inputs["proj_w"]),
        "w_fc1": tr(inputs["fc1_w"]), "w_fc2": tr(inputs["fc2_w"]),
        "sel12": sel, "one8": one8, "m01": m01, "ident": ident,
    }
    nc = _get_nc()
    in_maps = [dict(shared, x=np.ascontiguousarray(x[b].astype(bf)))
               for b in range(B)]
    res = run_bass_kernel_spmd(nc, in_maps, core_ids=list(range(B)),
                               trace=bool(int(os.environ.get("KTRACE", "0"))))
    out = np.stack([res.results[b]["out"] for b in range(B)], axis=0)
    kernel.last_results = res
    return out
